# revision 36
# baseline (speedup 1.0000x reference)
"""Fused multi-head attention kernel for Trainium2, SPMD over 8 NeuronCores.

Problem: nn_MultiHeadAttention (B=4, T=2048, C=512, H=8 heads, Dh=64).
  qkv = x @ W_attn + b_attn ; split q,k,v ; per-head softmax(q k^T / 8) v ;
  out = y @ W_out + b_out

Sharding: core c handles batch b = c//2 and heads hh*4..hh*4+3 (hh = c%2).
Each core computes a partial out-projection over its 4 heads' channels;
the host sums the two partials per batch and adds b_out.

Device-side layout is fully "transposed" (token axis on the free dim):
  xT [C, T] -> qT,kT [64h, T] (per head on partitions 0..63/64..127),
  v in natural [T, 256] layout padded with a ones column per head,
  scoresT [kpos, qpos] tiles -> exp on ScalarE -> AV matmuls give
  yT [d, qpos] with an extra row = softmax denominator (ones-column trick).
Softmax skips max-subtraction: scores ~ N(0,1), |s|max < ~10, safe in fp32.
Matmul inputs are bf16 (PSUM accumulation fp32); exp input fp32 from PSUM.
"""

import sys

if "/opt/trn_rl_repo" not in sys.path:
    sys.path.insert(0, "/opt/trn_rl_repo")

import numpy as np
import ml_dtypes

B, T_FULL, C = 4, 2048, 512
H, DH = 8, 64
HPC = 4  # heads per core
N_CORES = 8

_prog_cache = {}


def build_nc(T=T_FULL):
    import concourse.bass as bass
    import concourse.tile as tile
    from concourse import bacc, mybir
    from concourse.bass import ts

    f32 = mybir.dt.float32
    bf16 = mybir.dt.bfloat16
    # attention-probability dtype: fp16 (11-bit mantissa) is ~16x more precise
    # than bf16 for exp outputs, same 1 cyc/row PE rate; exp(s/8 - 2) keeps the
    # largest value ~e^6 even for outlier scores, far from fp16's 65504 max.
    f16 = mybir.dt.float16
    EXP_SHIFT = -2.0

    KT = T // 128         # kpos chunks
    NQ = max(1, T // 512) # q tiles of 512
    QW = min(T, 512)      # q tile width
    CH = HPC * DH         # 256 channels per core per q/k/v

    def pbcast(ap, nparts):
        """Partition-broadcast a 1-D (free-only) AP to [nparts, ...] for DMA."""
        return bass.AP(
            tensor=ap.tensor, offset=ap.offset, ap=[[0, nparts]] + list(ap.ap)
        )

    def pbcast2(ap, nparts):
        """Same for a [1, N] AP: replace the partition dim with a stride-0
        broadcast so a DMA can replicate one SBUF row across partitions."""
        return bass.AP(
            tensor=ap.tensor, offset=ap.offset, ap=[[0, nparts]] + list(ap.ap)[1:]
        )

    # Bacc (not raw Bass): its finalize() runs move_matmul_waits_to_ldweights +
    # generate_event_semaphores, legalizing the TRN2 1-wait-per-instruction limit.
    nc = bacc.Bacc("TRN2")

    # all inputs are pre-swizzled on the host into partition-major layouts so
    # every DMA descriptor is 2-4KB contiguous (512B descriptors measured
    # ~5x under DMA-ring peak and 1.2-2.5us of descgen per instruction)
    xT = nc.dram_tensor("xT", [128, 4, 4, QW], bf16, kind="ExternalInput")
    wq = nc.dram_tensor("wq", [128, 4, CH], bf16, kind="ExternalInput")
    wk = nc.dram_tensor("wk", [128, 4, CH], bf16, kind="ExternalInput")
    wv = nc.dram_tensor("wv", [128, 4, CH], bf16, kind="ExternalInput")
    bq = nc.dram_tensor("bq", [128, 2], f32, kind="ExternalInput")
    bk = nc.dram_tensor("bk", [128, 2], f32, kind="ExternalInput")
    bv = nc.dram_tensor("bv", [CH], f32, kind="ExternalInput")
    wo = nc.dram_tensor("wo", [128, 2, C], bf16, kind="ExternalInput")
    # partial (per-core) contribution; host sums core pairs in f32, so bf16
    # is plenty and halves the writeback bytes
    out = nc.dram_tensor("out", [T, C], bf16, kind="ExternalOutput")

    with tile.TileContext(nc) as tc:
        with (
            tc.tile_pool(name="consts", bufs=1) as consts,
            tc.tile_pool(name="ps_sc", bufs=2, space="PSUM") as ps_sc,
            tc.tile_pool(name="ps512", bufs=4, space="PSUM") as ps512,
            tc.tile_pool(name="expp", bufs=10) as expp,
            tc.tile_pool(name="rsm", bufs=4) as rsm,
            tc.tile_pool(name="rbp", bufs=4) as rbp,
            tc.tile_pool(name="outp", bufs=3) as outp,
        ):
            # ---- constant loads ----
            # Two HWDGE queues (SP="sync", Activation="scalar") descgen in
            # parallel; DMAs are ordered by first-use so the first projection
            # chains start ~2us in instead of waiting for the whole 3MB.
            exp_bias = consts.tile([128, 1], f32)
            nc.vector.memset(exp_bias[:], EXP_SHIFT)
            # warmup activation: forces the ~2.7us ACT table load to run at
            # t=0, before the scalar-queue DMA descgens and first real exp
            warm = consts.tile([128, 1], f32)
            nc.scalar.activation(
                out=warm[:],
                in_=exp_bias[:],
                func=mybir.ActivationFunctionType.Exp,
            )
            # sync queue carries the compute-critical loads in need order;
            # the scalar queue takes the rest
            wk_sb = consts.tile([128, 4, CH], bf16)
            nc.sync.dma_start(wk_sb[:], wk[:])
            # xT in 4 token chunks: chain nt / v-chunk g only waits on its own
            # chunk's DMA (tile framework tracks region-level overlap)
            xT_sb = consts.tile([128, 4, 4, QW], bf16)
            nc.sync.dma_start(xT_sb[:, 0], xT[:, 0])
            wq_sb = consts.tile([128, 4, CH], bf16)
            nc.sync.dma_start(wq_sb[:], wq[:])
            for _c in range(1, 4):
                nc.sync.dma_start(xT_sb[:, _c], xT[:, _c])
            bq_sb = consts.tile([128, 2], f32)
            nc.scalar.dma_start(bq_sb[:], bq[:])
            bk_sb = consts.tile([128, 2], f32)
            nc.scalar.dma_start(bk_sb[:], bk[:])
            bv_sb = consts.tile([128, CH], f32)
            nc.scalar.dma_start(bv_sb[:], pbcast(bv[:], 128))
            wv_sb = consts.tile([128, 4, CH], bf16)
            nc.scalar.dma_start(wv_sb[:], wv[:])
            # head-PAIR rows: wo2_sb[:, pr, :] = W_out rows for heads 2pr,
            # 2pr+1 (channel = h*64+d), matching the packed y_pair layout so
            # the out-proj contracts K=128 (2 heads) per matmul
            wo2_sb = consts.tile([128, 2, C], bf16)
            nc.scalar.dma_start(wo2_sb[:], wo[:])
            # Pre-touch DMA-loaded tiles on DVE: tensor_scalar/tensor_tensor
            # instructions have too few sync-wait slots to wait on both a PE
            # semaphore and a DMA semaphore; a cheap DVE read here makes the
            # DVE clock observe the DMA completion so later ops need only the
            # PE wait (walrus NCC_INLA001 "Too many sync wait commands").
            touch = consts.tile([128, 8], f32)
            nc.vector.tensor_copy(out=touch[:, 0:2], in_=bq_sb[:])
            nc.vector.tensor_copy(out=touch[:, 2:4], in_=bk_sb[:])
            nc.vector.tensor_copy(out=touch[:, 4:5], in_=bv_sb[:, 0:1])
            # Same trick for the PE clock: a dummy ldweights per DMA-loaded
            # matmul input makes PE observe the DMA queues once, so real
            # matmuls never carry a DMA wait on top of their compute waits.
            # The PE queue is in-order, so these are STAGGERED by DMA arrival
            # (first-needed first); late arrivals (wv, xT chunks 2-3, wo2)
            # are touched from the drip-feed backlog / main loop instead of
            # head-blocking the first projection chains here.
            nc.tensor.ldweights(wk_sb[:, 0, 0:128])
            nc.tensor.ldweights(wq_sb[:, 0, 0:128])
            nc.tensor.ldweights(xT_sb[:, 0, 0, 0:128])
            # ones row for the K=1 broadcast matmul in the softmax division
            ones64 = consts.tile([1, DH], f16)
            nc.vector.memset(ones64[:], 1.0)

            # ---- computed tensors ----
            # group 0 (heads 0/1) is split into column tiles so the first
            # scores matmul only waits on 3 projection chains, not 8:
            # qT0h[qh] covers the qpos half a phase reads; kTn[nt] one 512 col
            # chunk of kT. Group 1 stays monolithic (it is drip-fed early).
            HW2_ = T // 2
            qT0h = [
                consts.tile([128, HW2_], bf16, tag=f"qT0h{i}", name=f"qT0h{i}")
                for i in range(2)
            ]
            kTn = [
                consts.tile([128, QW], bf16, tag=f"kTn{i}", name=f"kTn{i}")
                for i in range(NQ)
            ]
            qT1 = consts.tile([128, T], bf16, tag="qT1", name="qT1")
            kT1 = consts.tile([128, T], bf16, tag="kT1", name="kT1")
            # v (natural layout) padded with ones column: [128, KT, HPC, 65]
            v_ones = consts.tile([128, KT, HPC, DH + 1], f16)
            nc.vector.memset(v_ones[:, :, :, DH : DH + 1], 1.0)
            # yT packed per head-PAIR (head 2p at rows 0-63, 2p+1 at 64-127)
            # so out-proj matmuls contract K=128; odd heads' divisions write a
            # base-0 staging tile (DVE can't shift partitions) that a small
            # SBUF->SBUF DMA moves into the pair tile's high half
            y_pair = {
                (p, qh): consts.tile(
                    [128, T // 2], bf16, tag=f"yp{p}_{qh}", name=f"yp{p}_{qh}"
                )
                for p in range(2)
                for qh in range(2)
            }
            yTodd = {
                (h, qh): consts.tile(
                    [64, T // 2], bf16, tag=f"yo{h}_{qh}", name=f"yo{h}_{qh}"
                )
                for h in (1, 3)
                for qh in range(2)
            }

            # ---- Phase 1: QKV projection ----
            def qk_chain(m, w_sb, b_sb, dst_ap, nt):
                pt = ps512.tile([128, 512], f32, tag="mm512", name=f"qk_{m}_{nt}")
                for kt in range(4):
                    nc.tensor.matmul(
                        pt[:, :QW],
                        w_sb[:, kt, m * 128 : (m + 1) * 128],
                        xT_sb[:, nt, kt, :QW],
                        start=(kt == 0),
                        stop=(kt == 3),
                    )
                nc.vector.tensor_scalar_add(
                    out=dst_ap,
                    in0=pt[:, :QW],
                    scalar1=b_sb[:, m : m + 1],
                )

            def q0_dst(nt):
                qh = (nt * QW) // HW2_
                off = (nt * QW) % HW2_
                return qT0h[qh][:, off : off + QW]

            def v_chunk(g):
                pt = ps512.tile([128, 512], f32, tag="mm512", name=f"v_{g}")
                for kt in range(4):
                    nc.tensor.matmul(
                        pt[:, :CH],
                        xT_sb[:, g // 4, kt, (g % 4) * 128 : (g % 4) * 128 + 128],
                        wv_sb[:, kt, :],
                        start=(kt == 0),
                        stop=(kt == 3),
                    )
                nc.vector.tensor_add(
                    out=v_ones[:, g, :, 0:DH],
                    in0=pt[:, :CH].rearrange("p (h d) -> p h d", h=HPC),
                    in1=bv_sb[:].rearrange("p (h d) -> p h d", h=HPC),
                )

            # Pre-loop: only what scores step 0 needs — kT chunk 0 and the
            # first qpos-half of qT, both for group 0. Everything else is
            # drip-fed into the early loop steps (2 chains + 2 v-chunks per
            # step) while the AV PSUM-slot demand is still zero.
            nq_half = max(1, HW2_ // QW)  # q chains per qh half
            qk_chain(0, wk_sb, bk_sb, kTn[0][:, :QW], 0)
            for nt in range(nq_half):
                if nt > 0:  # PE observes xT chunk nt's DMA before using it
                    nc.tensor.ldweights(xT_sb[:, nt, 0, 0:128])
                qk_chain(0, wq_sb, bq_sb, q0_dst(nt), nt)
            # late-arriving DMAs (xT chunks 2-3, wv land ~5-9us in) are
            # observed by a dummy ldweights placed at the LAST moment before
            # their first user, so the in-order PE stream never head-blocks
            proj_backlog = []
            for nt in range(1, NQ):  # k chunks in need-order (g = 4*nt)
                if nt >= 2:
                    proj_backlog.append(
                        lambda nt=nt: nc.tensor.ldweights(
                            xT_sb[:, nt, 0, 0:128]
                        )
                    )
                proj_backlog.append(
                    lambda nt=nt: qk_chain(0, wk_sb, bk_sb, kTn[nt][:, :QW], nt)
                )
            for nt in range(nq_half, NQ):
                proj_backlog.append(
                    lambda nt=nt: qk_chain(0, wq_sb, bq_sb, q0_dst(nt), nt)
                )
            for nt in range(NQ):
                proj_backlog.append(
                    lambda nt=nt: qk_chain(1, wq_sb, bq_sb, qT1[:, ts(nt, QW)], nt)
                )
                proj_backlog.append(
                    lambda nt=nt: qk_chain(1, wk_sb, bk_sb, kT1[:, ts(nt, QW)], nt)
                )
            # wv lands ~4us in, before the drip loop's first v_chunk; touch it
            # here (not in the backlog — the backlog MUST drain by step 8,
            # when the AV accumulators take every ps512 slot)
            nc.tensor.ldweights(wv_sb[:, 0, 0:128])
            v_backlog = [lambda g=g: v_chunk(g) for g in range(KT)]

            # ---- Phase 2: attention ----
            # Head-PAIR processing: heads hA=2p (partitions 0-63) and hB=2p+1
            # (64-127) issue adjacent row-tiled matmuls that run concurrently
            # on the PE array, writing disjoint column ranges of one scores
            # PSUM tile [128, T]: cols [0, T/2) = hA's qpos half, [T/2, T) =
            # hB's same qpos half. Each pair is covered in 2 "qh" phases.
            # AV matmuls lag 2 steps behind scores/exp (lag-2 pipeline).
            # bank-disjointness of the concurrent head-pair matmuls requires
            # each head's column range to cover whole PSUM banks (>=512 f32)
            assert T >= 1024, "pair-packed scores need T/2 >= 512 (PSUM bank)"
            HW2 = T // 2              # qpos width per head per scores tile
            QW2 = min(512, HW2)       # AV / division chunk width
            NQS = HW2 // QW2          # AV chains per head per phase
            expT = {}                 # step -> sbuf tile [128, T]
            av_ps = {}                # (h, qs) -> psum tile
            pending_div = []          # deferred division finishers
            next_mq = [0]             # out-proj chunks emitted in-loop
            NPH = 2 * 2               # pairs * qh phases
            NSTEP = NPH * KT

            # Phase order (p,qh): (0,0) (1,0) (0,1) (1,1) — both pairs finish
            # qh=0 by mid-kernel, so the first half of the out-projection (and
            # its DMA writeback) overlaps the qh=1 attention phases.
            def decode(s):
                ph, g = divmod(s, KT)
                qh, p = divmod(ph, 2)
                return p, qh, g

            # AV-step retiming. Phase 0 delays AV to step 8 (catch-up 2/step)
            # so the early steps' PSUM slots are free for the interleaved
            # v-projection / qk group-1 chains; later phases delay their first
            # AV allocations to +6 so the previous phase's division finishers
            # (all popped by +3) have released every AV slot — allocating
            # earlier would let a PE matmul wait on a slot whose release is
            # behind it in the PE stream (deadlock).
            av_sched = {}
            for _ph in range(NPH):
                for _g in range(KT):
                    _aq = _ph * KT + _g
                    if _ph == 0:
                        _run = max(_g + 2, 8 + _g // 2)
                    else:
                        _run = _ph * KT + max(_g + 2, 4 + _g // 2)
                    av_sched.setdefault(_run, []).append(_aq)

            def av_step(s):
                p, qh, g = decode(s)
                for h2 in range(2):
                    h = 2 * p + h2
                    for qs in range(NQS):
                        if g == 0:
                            av_ps[(h, qs)] = ps512.tile(
                                [128, 512], f32, tag="mm512", name=f"av_{s}_{h2}_{qs}"
                            )
                        nc.tensor.matmul(
                            av_ps[(h, qs)][: DH + 1, :QW2],
                            v_ones[:, g, h, :],
                            expT[s][:, h2 * HW2 + qs * QW2 : h2 * HW2 + (qs + 1) * QW2],
                            start=(g == 0),
                            stop=(g == KT - 1),
                        )
                if g == KT - 1:
                    emit_divs(p, qh)

            def emit_divs(p, qh):
                final = p == 1 and qh == 1
                # Reciprocals now (DVE, off critical path), then a DMA
                # partition-broadcast replicates 1/denom across 64 partitions
                # (DMA rings are idle mid-attention; the old K=1 PE broadcast
                # + DVE copy cost ~430ns PE + ~620ns DVE per chain)
                for h2 in range(2):
                    h = 2 * p + h2
                    for qs in range(NQS):
                        av = av_ps.pop((h, qs))
                        # custom-DVE ops corrupt data when reading PSUM
                        # directly — bounce the denominator row to SBUF
                        den = rsm.tile([1, 512], f32, tag="den", name=f"dn_{h}_{qh}_{qs}")
                        # final phase: den bounce on the then-idle ACT engine
                        # so the tail's serial DVE pipeline halves
                        if final:
                            nc.scalar.copy(out=den[:, :QW2], in_=av[DH : DH + 1, :QW2])
                        else:
                            nc.vector.tensor_copy(
                                out=den[:, :QW2], in_=av[DH : DH + 1, :QW2]
                            )
                        rf = rsm.tile([1, 512], f32, tag="rf", name=f"rf_{h}_{qh}_{qs}")
                        nc.vector.reciprocal_approx_fast(
                            out=rf[:, :QW2], in_=den[:, :QW2]
                        )
                        rb = rbp.tile([64, 512], f32)
                        # replicate across partitions on the (idle) GpSimd
                        # engine: SBUF APs can't stride-0 the partition dim,
                        # so a DMA can't do this and the PE K=1 matmul
                        # broadcast cost ~430ns PE + ~620ns DVE per chain
                        nc.gpsimd.partition_broadcast(rb[:, :QW2], rf[0:1, :QW2])
                        pending_div.append((h, qh, qs, rb, av))

            def finish_div(h, qh, qs, rb, av):
                # a 1-element DVE touch observes the broadcast-DMA completion
                # so the multiply itself needs only the PE wait
                nc.vector.tensor_copy(out=touch[0:1, 5:6], in_=rb[0:1, 0:1])
                col = qs * QW2
                if h % 2 == 0:
                    dst = y_pair[(h // 2, qh)][0:64, col : col + QW2]
                else:
                    dst = yTodd[(h, qh)][:, col : col + QW2]
                nc.vector.tensor_mul(
                    out=dst,
                    in0=av[0:DH, :QW2],
                    in1=rb[:, :QW2],
                )
                if h % 2 == 1:
                    # move the odd head's slice into the pair tile's high half
                    nc.sync.dma_start(
                        y_pair[(h // 2, qh)][64:128, col : col + QW2],
                        yTodd[(h, qh)][:, col : col + QW2],
                    )

            # ---- out-projection chunk emitter (partial, this core's heads) ----
            # PSUM comes from the scores pool (same tag = same slots); early
            # chunks are interleaved into the qh=1 attention phases.
            def emit_outproj(mq, dma_eng=None, act_copy=False):
                opt = ps_sc.tile([128, HW2], f32, tag="spt", name=f"op_{mq}")
                mqh, mcol = (mq * 128) // HW2, (mq * 128) % HW2
                for pp in range(2):
                    nc.tensor.matmul(
                        opt[:, :512],
                        y_pair[(pp, mqh)][:, mcol : mcol + 128],
                        wo2_sb[:, pp, :],
                        start=(pp == 0),
                        stop=(pp == 1),
                    )
                ot = outp.tile([128, 512], bf16)
                # tail chunks copy on the (then-idle) ACT engine so the DVE
                # doesn't pace the PSUM slot recycling; in-loop chunks use
                # DVE (ACT is the bottleneck mid-attention)
                if act_copy:
                    nc.scalar.copy(out=ot[:], in_=opt[:, :512])
                else:
                    nc.vector.tensor_copy(out=ot[:], in_=opt[:, :512])
                (dma_eng or nc.sync).dma_start(out[ts(mq, 128), :], ot[:])

            # Two scores PSUM tiles per step (one per head of the pair, 2
            # banks each) from a bufs=2 pool: head-A's scores of step s+1
            # only wait on exp-A(s) — exp and scores ping-pong with full
            # ACT overlap instead of serializing on one tile.
            for s in range(NSTEP):
                p, qh, g = decode(s)
                # all qh=0 divisions land by step 35 (2 phases + finisher
                # drain), so the first 8 out-proj chunks interleave into the
                # qh=1 phases, one per 3 steps, at the step TOP: the scores
                # pool slot it takes then has a full step of slack before
                # the next scores allocation needs it
                if s == 16:  # PE observes wo2's DMA well before out-proj
                    nc.tensor.ldweights(wo2_sb[:, 0, 0:128])
                if s >= 38 and (s - 38) % 3 == 0 and next_mq[0] < 8:
                    emit_outproj(next_mq[0])
                    next_mq[0] += 1
                if s < 8:  # drip-feed remaining projection work
                    for _ in range(2):
                        if v_backlog:
                            v_backlog.pop(0)()
                        if proj_backlog:
                            proj_backlog.pop(0)()
                # The pair's scores MMs are INTERLEAVED (A,B,A,B): heads
                # A/B occupy disjoint PE row-halves (tile_position auto
                # (0,0)/(64,0)), so each MM's weight load hoists over the
                # other head's in-flight MM instead of serializing
                et = expp.tile([128, T], f16)
                if p == 0:
                    knt, koff = (g * 128) // QW, (g * 128) % QW
                spts = [
                    ps_sc.tile([128, HW2], f32, tag="spt", name=f"spt_{s}_{_h}")
                    for _h in range(2)
                ]
                for qs in range(NQS):
                    for h2 in range(2):
                        hb = h2 * 64
                        if p == 0:
                            lhsT = kTn[knt][hb : hb + 64, koff : koff + 128]
                            rhs = qT0h[qh][
                                hb : hb + 64, qs * QW2 : (qs + 1) * QW2
                            ]
                        else:
                            lhsT = kT1[hb : hb + 64, ts(g, 128)]
                            rhs = qT1[
                                hb : hb + 64,
                                qh * HW2 + qs * QW2 : qh * HW2 + (qs + 1) * QW2,
                            ]
                        nc.tensor.matmul(
                            spts[h2][:, qs * QW2 : (qs + 1) * QW2],
                            lhsT,
                            rhs,
                            start=True,
                            stop=True,
                        )
                for h2 in range(2):
                    nc.scalar.activation(
                        out=et[:, h2 * HW2 : (h2 + 1) * HW2],
                        in_=spts[h2][:],
                        func=mybir.ActivationFunctionType.Exp,
                        bias=exp_bias[:],
                        scale=1.0 / 8.0,
                    )
                expT[s] = et
                # AV matmuls AFTER the step's scores: the exp->next-scores
                # PSUM ping-pong then overlaps the AV block instead of
                # serializing behind it (scores(s+1) needs exp(s) retired;
                # placing AV between them absorbs the exp latency)
                for aq in av_sched.pop(s, []):
                    av_step(aq)
                # division finishers at the END of the step: their broadcast
                # matmul waits on a DVE reciprocal, and at the head of the
                # step it would stall the PE stream ahead of independent
                # scores/AV work (measured 2.6us ACT gaps per phase boundary)
                for _ in range(2):
                    if pending_div:
                        finish_div(*pending_div.pop(0))
                if s == 8:
                    # anything not drip-fed (shouldn't happen at T=2048)
                    while v_backlog:
                        v_backlog.pop(0)()
                    while proj_backlog:
                        proj_backlog.pop(0)()
            for s in sorted(av_sched):
                for _ in range(2):
                    if pending_div:
                        finish_div(*pending_div.pop(0))
                for aq in av_sched[s]:
                    av_step(aq)
            av_sched.clear()

            # ---- tail: remaining divisions + second-half out-projection ----
            # chunks 8-11 read only qs=0 columns, 12-15 only qs=1: pop the
            # matching finishers just ahead (finishers MUST precede their
            # reader chunks in the in-order DVE stream or it deadlocks)
            base = next_mq[0]
            for want_qs in range(NQS):
                for d in [d for d in pending_div if d[2] == want_qs]:
                    finish_div(*d)
                pending_div = [d for d in pending_div if d[2] != want_qs]
                for i in range(4):
                    mq = base + want_qs * 4 + i
                    emit_outproj(
                        mq,
                        dma_eng=nc.scalar if mq % 2 else nc.sync,
                        act_copy=(mq % 2 == 1),
                    )
            while pending_div:
                finish_div(*pending_div.pop(0))

    nc.finalize()
    return nc


def make_in_maps(x, W_attn, b_attn, W_out):
    """Shard full inputs across 8 cores: core c = (batch c//2, head-half c%2).

    Everything is pre-swizzled into partition-major layouts so device DMA
    descriptors are 2-4KB contiguous per partition:
      xT  [p, chunk, ko, t]  (channel = ko*128 + p, token = chunk*512 + t)
      w*  [p, ko, m]         (input channel = ko*128 + p)
      b*  [p, o]             (channel = o*128 + p)
      wo  [p, pr, n]         (y channel = pr*128 + p)
    """
    bf = ml_dtypes.bfloat16
    in_maps = []
    for c in range(N_CORES):
        b, hh = divmod(c, 2)
        sl = slice(hh * HPC * DH, (hh + 1) * HPC * DH)  # channel slice (256)

        def wsw(w):  # [512, 256] -> [128, 4, 256]
            return np.ascontiguousarray(
                w.reshape(4, 128, HPC * DH).transpose(1, 0, 2)
            ).astype(bf)

        xt = x[b].T  # [C, T]
        in_maps.append(
            {
                "xT": np.ascontiguousarray(
                    xt.reshape(4, 128, 4, 512).transpose(1, 2, 0, 3)
                ).astype(bf),
                "wq": wsw(W_attn[:, 0 * C :][:, sl]),
                "wk": wsw(W_attn[:, 1 * C :][:, sl]),
                "wv": wsw(W_attn[:, 2 * C :][:, sl]),
                "bq": np.ascontiguousarray(
                    b_attn[0 * C :][sl].reshape(2, 128).T, dtype=np.float32
                ),
                "bk": np.ascontiguousarray(
                    b_attn[1 * C :][sl].reshape(2, 128).T, dtype=np.float32
                ),
                "bv": np.ascontiguousarray(b_attn[2 * C :][sl], dtype=np.float32),
                "wo": np.ascontiguousarray(
                    W_out[sl, :].reshape(2, 128, C).transpose(1, 0, 2)
                ).astype(bf),
            }
        )
    return in_maps


def kernel(x, W_attn, b_attn, W_out, b_out, _trace=False):
    from concourse.bass_utils import run_bass_kernel_spmd

    x = np.asarray(x, dtype=np.float32)
    W_attn = np.asarray(W_attn, dtype=np.float32)
    b_attn = np.asarray(b_attn, dtype=np.float32)
    W_out = np.asarray(W_out, dtype=np.float32)
    b_out = np.asarray(b_out, dtype=np.float32)

    key = T_FULL
    if key not in _prog_cache:
        _prog_cache[key] = build_nc(T_FULL)
    nc = _prog_cache[key]

    in_maps = make_in_maps(x, W_attn, b_attn, W_out)
    res = run_bass_kernel_spmd(nc, in_maps, list(range(N_CORES)), trace=_trace)

    out = np.empty((B, T_FULL, C), dtype=np.float32)
    for b in range(B):
        out[b] = (
            res.results[2 * b]["out"].astype(np.float32)
            + res.results[2 * b + 1]["out"].astype(np.float32)
            + b_out
        )
    if _trace:
        kernel.last_exec_time_ns = res.exec_time_ns
        kernel.last_results = res
    return out



# revision 37
# speedup vs baseline: 1.0577x; 1.0577x over previous
"""Fused multi-head attention kernel for Trainium2, SPMD over 8 NeuronCores.

Problem: nn_MultiHeadAttention (B=4, T=2048, C=512, H=8 heads, Dh=64).
  qkv = x @ W_attn + b_attn ; split q,k,v ; per-head softmax(q k^T / 8) v ;
  out = y @ W_out + b_out

Sharding: core c handles batch b = c//2 and heads hh*4..hh*4+3 (hh = c%2).
Each core computes a partial out-projection over its 4 heads' channels;
the host sums the two partials per batch and adds b_out.

Device-side layout is fully "transposed" (token axis on the free dim):
  xT [C, T] -> qT,kT [64h, T] (per head on partitions 0..63/64..127),
  v in natural [T, 256] layout padded with a ones column per head,
  scoresT [kpos, qpos] tiles -> exp on ScalarE -> AV matmuls give
  yT [d, qpos] with an extra row = softmax denominator (ones-column trick).
Softmax skips max-subtraction: scores ~ N(0,1), |s|max < ~10, safe in fp32.
Matmul inputs are bf16 (PSUM accumulation fp32); exp input fp32 from PSUM.
"""

import sys

if "/opt/trn_rl_repo" not in sys.path:
    sys.path.insert(0, "/opt/trn_rl_repo")

import numpy as np
import ml_dtypes

B, T_FULL, C = 4, 2048, 512
H, DH = 8, 64
HPC = 4  # heads per core
N_CORES = 8

_prog_cache = {}


def build_nc(T=T_FULL):
    import concourse.bass as bass
    import concourse.tile as tile
    from concourse import bacc, mybir
    from concourse.bass import ts

    f32 = mybir.dt.float32
    bf16 = mybir.dt.bfloat16
    # attention-probability dtype: fp16 (11-bit mantissa) is ~16x more precise
    # than bf16 for exp outputs, same 1 cyc/row PE rate; exp(s/8 - 2) keeps the
    # largest value ~e^6 even for outlier scores, far from fp16's 65504 max.
    f16 = mybir.dt.float16
    EXP_SHIFT = -2.0

    KT = T // 128         # kpos chunks
    NQ = max(1, T // 512) # q tiles of 512
    QW = min(T, 512)      # q tile width
    CH = HPC * DH         # 256 channels per core per q/k/v

    def pbcast(ap, nparts):
        """Partition-broadcast a 1-D (free-only) AP to [nparts, ...] for DMA."""
        return bass.AP(
            tensor=ap.tensor, offset=ap.offset, ap=[[0, nparts]] + list(ap.ap)
        )

    def pbcast2(ap, nparts):
        """Same for a [1, N] AP: replace the partition dim with a stride-0
        broadcast so a DMA can replicate one SBUF row across partitions."""
        return bass.AP(
            tensor=ap.tensor, offset=ap.offset, ap=[[0, nparts]] + list(ap.ap)[1:]
        )

    # Bacc (not raw Bass): its finalize() runs move_matmul_waits_to_ldweights +
    # generate_event_semaphores, legalizing the TRN2 1-wait-per-instruction limit.
    nc = bacc.Bacc("TRN2")

    # all inputs are pre-swizzled on the host into partition-major layouts so
    # every DMA descriptor is 2-4KB contiguous (512B descriptors measured
    # ~5x under DMA-ring peak and 1.2-2.5us of descgen per instruction)
    xT = nc.dram_tensor("xT", [128, 4, 4, QW], bf16, kind="ExternalInput")
    wq = nc.dram_tensor("wq", [128, 4, CH], bf16, kind="ExternalInput")
    wk = nc.dram_tensor("wk", [128, 4, CH], bf16, kind="ExternalInput")
    wv = nc.dram_tensor("wv", [128, 4, CH], bf16, kind="ExternalInput")
    bq = nc.dram_tensor("bq", [128, 2], f32, kind="ExternalInput")
    bk = nc.dram_tensor("bk", [128, 2], f32, kind="ExternalInput")
    bv = nc.dram_tensor("bv", [CH], f32, kind="ExternalInput")
    wo = nc.dram_tensor("wo", [128, 2, C], bf16, kind="ExternalInput")
    # partial (per-core) contribution; host sums core pairs in f32, so bf16
    # is plenty and halves the writeback bytes
    out = nc.dram_tensor("out", [T, C], bf16, kind="ExternalOutput")

    with tile.TileContext(nc) as tc:
        with (
            tc.tile_pool(name="consts", bufs=1) as consts,
            tc.tile_pool(name="ps_sc", bufs=2, space="PSUM") as ps_sc,
            tc.tile_pool(name="ps512", bufs=4, space="PSUM") as ps512,
            tc.tile_pool(name="expp", bufs=10) as expp,
            tc.tile_pool(name="rsm", bufs=4) as rsm,
            tc.tile_pool(name="rbp", bufs=4) as rbp,
            tc.tile_pool(name="outp", bufs=3) as outp,
        ):
            # ---- constant loads ----
            # Two HWDGE queues (SP="sync", Activation="scalar") descgen in
            # parallel; DMAs are ordered by first-use so the first projection
            # chains start ~2us in instead of waiting for the whole 3MB.
            exp_bias = consts.tile([128, 1], f32)
            nc.vector.memset(exp_bias[:], EXP_SHIFT)
            # warmup activation: forces the ~2.7us ACT table load to run at
            # t=0, before the scalar-queue DMA descgens and first real exp
            warm = consts.tile([128, 1], f32)
            nc.scalar.activation(
                out=warm[:],
                in_=exp_bias[:],
                func=mybir.ActivationFunctionType.Exp,
            )
            # sync queue carries the compute-critical loads in need order;
            # the scalar queue takes the rest
            wk_sb = consts.tile([128, 4, CH], bf16)
            nc.sync.dma_start(wk_sb[:], wk[:])
            # xT in 4 token chunks: chain nt / v-chunk g only waits on its own
            # chunk's DMA (tile framework tracks region-level overlap)
            xT_sb = consts.tile([128, 4, 4, QW], bf16)
            nc.sync.dma_start(xT_sb[:, 0], xT[:, 0])
            wq_sb = consts.tile([128, 4, CH], bf16)
            nc.sync.dma_start(wq_sb[:], wq[:])
            for _c in range(1, 4):
                nc.sync.dma_start(xT_sb[:, _c], xT[:, _c])
            bq_sb = consts.tile([128, 2], f32)
            nc.scalar.dma_start(bq_sb[:], bq[:])
            bk_sb = consts.tile([128, 2], f32)
            nc.scalar.dma_start(bk_sb[:], bk[:])
            bv_sb = consts.tile([128, CH], f32)
            nc.scalar.dma_start(bv_sb[:], pbcast(bv[:], 128))
            wv_sb = consts.tile([128, 4, CH], bf16)
            nc.scalar.dma_start(wv_sb[:], wv[:])
            # head-PAIR rows: wo2_sb[:, pr, :] = W_out rows for heads 2pr,
            # 2pr+1 (channel = h*64+d), matching the packed y_pair layout so
            # the out-proj contracts K=128 (2 heads) per matmul
            wo2_sb = consts.tile([128, 2, C], bf16)
            nc.scalar.dma_start(wo2_sb[:], wo[:])
            # Pre-touch DMA-loaded tiles on DVE: tensor_scalar/tensor_tensor
            # instructions have too few sync-wait slots to wait on both a PE
            # semaphore and a DMA semaphore; a cheap DVE read here makes the
            # DVE clock observe the DMA completion so later ops need only the
            # PE wait (walrus NCC_INLA001 "Too many sync wait commands").
            touch = consts.tile([128, 8], f32)
            nc.vector.tensor_copy(out=touch[:, 0:2], in_=bq_sb[:])
            nc.vector.tensor_copy(out=touch[:, 2:4], in_=bk_sb[:])
            nc.vector.tensor_copy(out=touch[:, 4:5], in_=bv_sb[:, 0:1])
            # Same trick for the PE clock: a dummy ldweights per DMA-loaded
            # matmul input makes PE observe the DMA queues once, so real
            # matmuls never carry a DMA wait on top of their compute waits.
            # The PE queue is in-order, so these are STAGGERED by DMA arrival
            # (first-needed first); late arrivals (wv, xT chunks 2-3, wo2)
            # are touched from the drip-feed backlog / main loop instead of
            # head-blocking the first projection chains here.
            nc.tensor.ldweights(wk_sb[:, 0, 0:128])
            nc.tensor.ldweights(wq_sb[:, 0, 0:128])
            nc.tensor.ldweights(xT_sb[:, 0, 0, 0:128])
            # ones row for the K=1 broadcast matmul in the softmax division
            ones64 = consts.tile([1, DH], f16)
            nc.vector.memset(ones64[:], 1.0)

            # ---- computed tensors ----
            # group 0 (heads 0/1) is split into column tiles so the first
            # scores matmul only waits on 3 projection chains, not 8:
            # qT0h[qh] covers the qpos half a phase reads; kTn[nt] one 512 col
            # chunk of kT. Group 1 stays monolithic (it is drip-fed early).
            HW2_ = T // 2
            qT0h = [
                consts.tile([128, HW2_], bf16, tag=f"qT0h{i}", name=f"qT0h{i}")
                for i in range(2)
            ]
            kTn = [
                consts.tile([128, QW], bf16, tag=f"kTn{i}", name=f"kTn{i}")
                for i in range(NQ)
            ]
            qT1 = consts.tile([128, T], bf16, tag="qT1", name="qT1")
            kT1 = consts.tile([128, T], bf16, tag="kT1", name="kT1")
            # v (natural layout) padded with ones column: [128, KT, HPC, 65]
            v_ones = consts.tile([128, KT, HPC, DH + 1], f16)
            nc.vector.memset(v_ones[:, :, :, DH : DH + 1], 1.0)
            # yT packed per head-PAIR (head 2p at rows 0-63, 2p+1 at 64-127)
            # so out-proj matmuls contract K=128; odd heads' divisions write a
            # base-0 staging tile (DVE can't shift partitions) that a small
            # SBUF->SBUF DMA moves into the pair tile's high half
            y_pair = {
                (p, qh): consts.tile(
                    [128, T // 2], bf16, tag=f"yp{p}_{qh}", name=f"yp{p}_{qh}"
                )
                for p in range(2)
                for qh in range(2)
            }
            yTodd = {
                (h, qh): consts.tile(
                    [64, T // 2], bf16, tag=f"yo{h}_{qh}", name=f"yo{h}_{qh}"
                )
                for h in (1, 3)
                for qh in range(2)
            }

            # ---- Phase 1: QKV projection ----
            def qk_chain(m, w_sb, b_sb, dst_ap, nt):
                pt = ps512.tile([128, 512], f32, tag="mm512", name=f"qk_{m}_{nt}")
                for kt in range(4):
                    nc.tensor.matmul(
                        pt[:, :QW],
                        w_sb[:, kt, m * 128 : (m + 1) * 128],
                        xT_sb[:, nt, kt, :QW],
                        start=(kt == 0),
                        stop=(kt == 3),
                    )
                nc.vector.tensor_scalar_add(
                    out=dst_ap,
                    in0=pt[:, :QW],
                    scalar1=b_sb[:, m : m + 1],
                )

            def q0_dst(nt):
                qh = (nt * QW) // HW2_
                off = (nt * QW) % HW2_
                return qT0h[qh][:, off : off + QW]

            def v_chunk(g):
                pt = ps512.tile([128, 512], f32, tag="mm512", name=f"v_{g}")
                for kt in range(4):
                    nc.tensor.matmul(
                        pt[:, :CH],
                        xT_sb[:, g // 4, kt, (g % 4) * 128 : (g % 4) * 128 + 128],
                        wv_sb[:, kt, :],
                        start=(kt == 0),
                        stop=(kt == 3),
                    )
                nc.vector.tensor_add(
                    out=v_ones[:, g, :, 0:DH],
                    in0=pt[:, :CH].rearrange("p (h d) -> p h d", h=HPC),
                    in1=bv_sb[:].rearrange("p (h d) -> p h d", h=HPC),
                )

            # Pre-loop: only what scores step 0 needs — kT chunk 0 and the
            # first qpos-half of qT, both for group 0. Everything else is
            # drip-fed into the early loop steps (2 chains + 2 v-chunks per
            # step) while the AV PSUM-slot demand is still zero.
            nq_half = max(1, HW2_ // QW)  # q chains per qh half
            qk_chain(0, wk_sb, bk_sb, kTn[0][:, :QW], 0)
            for nt in range(nq_half):
                if nt > 0:  # PE observes xT chunk nt's DMA before using it
                    nc.tensor.ldweights(xT_sb[:, nt, 0, 0:128])
                qk_chain(0, wq_sb, bq_sb, q0_dst(nt), nt)
            # late-arriving DMAs (xT chunks 2-3, wv land ~5-9us in) are
            # observed by a dummy ldweights placed at the LAST moment before
            # their first user, so the in-order PE stream never head-blocks
            proj_backlog = []
            for nt in range(1, NQ):  # k chunks in need-order (g = 4*nt)
                if nt >= 2:
                    proj_backlog.append(
                        lambda nt=nt: nc.tensor.ldweights(
                            xT_sb[:, nt, 0, 0:128]
                        )
                    )
                proj_backlog.append(
                    lambda nt=nt: qk_chain(0, wk_sb, bk_sb, kTn[nt][:, :QW], nt)
                )
            for nt in range(nq_half, NQ):
                proj_backlog.append(
                    lambda nt=nt: qk_chain(0, wq_sb, bq_sb, q0_dst(nt), nt)
                )
            for nt in range(NQ):
                proj_backlog.append(
                    lambda nt=nt: qk_chain(1, wq_sb, bq_sb, qT1[:, ts(nt, QW)], nt)
                )
                proj_backlog.append(
                    lambda nt=nt: qk_chain(1, wk_sb, bk_sb, kT1[:, ts(nt, QW)], nt)
                )
            # wv lands ~4us in, before the drip loop's first v_chunk; touch it
            # here (not in the backlog — the backlog MUST drain by step 8,
            # when the AV accumulators take every ps512 slot)
            nc.tensor.ldweights(wv_sb[:, 0, 0:128])
            v_backlog = [lambda g=g: v_chunk(g) for g in range(KT)]

            # ---- Phase 2: attention ----
            # Head-PAIR processing: heads hA=2p (partitions 0-63) and hB=2p+1
            # (64-127) issue adjacent row-tiled matmuls that run concurrently
            # on the PE array, writing disjoint column ranges of one scores
            # PSUM tile [128, T]: cols [0, T/2) = hA's qpos half, [T/2, T) =
            # hB's same qpos half. Each pair is covered in 2 "qh" phases.
            # AV matmuls lag 2 steps behind scores/exp (lag-2 pipeline).
            # bank-disjointness of the concurrent head-pair matmuls requires
            # each head's column range to cover whole PSUM banks (>=512 f32)
            assert T >= 1024, "pair-packed scores need T/2 >= 512 (PSUM bank)"
            HW2 = T // 2              # qpos width per head per scores tile
            QW2 = min(512, HW2)       # AV / division chunk width
            NQS = HW2 // QW2          # AV chains per head per phase
            expT = {}                 # step -> sbuf tile [128, T]
            av_ps = {}                # (h, qs) -> psum tile
            pending_div = []          # deferred division finishers
            next_mq = [0]             # out-proj chunks emitted in-loop
            NPH = 2 * 2               # pairs * qh phases
            NSTEP = NPH * KT

            # Phase order (p,qh): (0,0) (1,0) (0,1) (1,1) — both pairs finish
            # qh=0 by mid-kernel, so the first half of the out-projection (and
            # its DMA writeback) overlaps the qh=1 attention phases.
            def decode(s):
                ph, g = divmod(s, KT)
                qh, p = divmod(ph, 2)
                return p, qh, g

            # AV-step retiming. Phase 0 delays AV to step 8 (catch-up 2/step)
            # so the early steps' PSUM slots are free for the interleaved
            # v-projection / qk group-1 chains; later phases delay their first
            # AV allocations to +6 so the previous phase's division finishers
            # (all popped by +3) have released every AV slot — allocating
            # earlier would let a PE matmul wait on a slot whose release is
            # behind it in the PE stream (deadlock).
            av_sched = {}
            for _ph in range(NPH):
                for _g in range(KT):
                    _aq = _ph * KT + _g
                    if _ph == 0:
                        _run = max(_g + 2, 8 + _g // 2)
                    else:
                        _run = _ph * KT + max(_g + 2, 4 + _g // 2)
                    av_sched.setdefault(_run, []).append(_aq)

            def av_step(s):
                p, qh, g = decode(s)
                for h2 in range(2):
                    h = 2 * p + h2
                    for qs in range(NQS):
                        if g == 0:
                            av_ps[(h, qs)] = ps512.tile(
                                [128, 512], f32, tag="mm512", name=f"av_{s}_{h2}_{qs}"
                            )
                        nc.tensor.matmul(
                            av_ps[(h, qs)][: DH + 1, :QW2],
                            v_ones[:, g, h, :],
                            expT[s][:, h2 * HW2 + qs * QW2 : h2 * HW2 + (qs + 1) * QW2],
                            start=(g == 0),
                            stop=(g == KT - 1),
                        )
                if g == KT - 1:
                    emit_divs(p, qh)

            def emit_divs(p, qh):
                # Reciprocals now (DVE, off critical path), then a DMA
                # partition-broadcast replicates 1/denom across 64 partitions
                # (DMA rings are idle mid-attention; the old K=1 PE broadcast
                # + DVE copy cost ~430ns PE + ~620ns DVE per chain)
                for h2 in range(2):
                    h = 2 * p + h2
                    for qs in range(NQS):
                        av = av_ps.pop((h, qs))
                        # custom-DVE ops corrupt data when reading PSUM
                        # directly — bounce the denominator row to SBUF
                        den = rsm.tile([1, 512], f32, tag="den", name=f"dn_{h}_{qh}_{qs}")
                        nc.vector.tensor_copy(
                            out=den[:, :QW2], in_=av[DH : DH + 1, :QW2]
                        )
                        rf = rsm.tile([1, 512], f32, tag="rf", name=f"rf_{h}_{qh}_{qs}")
                        nc.vector.reciprocal_approx_fast(
                            out=rf[:, :QW2], in_=den[:, :QW2]
                        )
                        rb = rbp.tile([64, 512], f32)
                        # replicate across partitions on the (idle) GpSimd
                        # engine: SBUF APs can't stride-0 the partition dim,
                        # so a DMA can't do this and the PE K=1 matmul
                        # broadcast cost ~430ns PE + ~620ns DVE per chain
                        nc.gpsimd.partition_broadcast(rb[:, :QW2], rf[0:1, :QW2])
                        pending_div.append((h, qh, qs, rb, av))

            def finish_div(h, qh, qs, rb, av):
                # a 1-element DVE touch observes the broadcast-DMA completion
                # so the multiply itself needs only the PE wait
                nc.vector.tensor_copy(out=touch[0:1, 5:6], in_=rb[0:1, 0:1])
                col = qs * QW2
                if h % 2 == 0:
                    dst = y_pair[(h // 2, qh)][0:64, col : col + QW2]
                else:
                    dst = yTodd[(h, qh)][:, col : col + QW2]
                nc.vector.tensor_mul(
                    out=dst,
                    in0=av[0:DH, :QW2],
                    in1=rb[:, :QW2],
                )
                if h % 2 == 1:
                    # move the odd head's slice into the pair tile's high half
                    nc.sync.dma_start(
                        y_pair[(h // 2, qh)][64:128, col : col + QW2],
                        yTodd[(h, qh)][:, col : col + QW2],
                    )

            # ---- out-projection chunk emitter (partial, this core's heads) ----
            # PSUM comes from the scores pool (same tag = same slots); early
            # chunks are interleaved into the qh=1 attention phases.
            def emit_outproj(mq, dma_eng=None, act_copy=False):
                opt = ps_sc.tile([128, HW2], f32, tag="spt", name=f"op_{mq}")
                mqh, mcol = (mq * 128) // HW2, (mq * 128) % HW2
                for pp in range(2):
                    nc.tensor.matmul(
                        opt[:, :512],
                        y_pair[(pp, mqh)][:, mcol : mcol + 128],
                        wo2_sb[:, pp, :],
                        start=(pp == 0),
                        stop=(pp == 1),
                    )
                ot = outp.tile([128, 512], bf16)
                # tail chunks copy on the (then-idle) ACT engine so the DVE
                # doesn't pace the PSUM slot recycling; in-loop chunks use
                # DVE (ACT is the bottleneck mid-attention)
                if act_copy:
                    nc.scalar.copy(out=ot[:], in_=opt[:, :512])
                else:
                    nc.vector.tensor_copy(out=ot[:], in_=opt[:, :512])
                (dma_eng or nc.sync).dma_start(out[ts(mq, 128), :], ot[:])

            # Two scores PSUM tiles per step (one per head of the pair, 2
            # banks each) from a bufs=2 pool: head-A's scores of step s+1
            # only wait on exp-A(s) — exp and scores ping-pong with full
            # ACT overlap instead of serializing on one tile.
            for s in range(NSTEP):
                p, qh, g = decode(s)
                # all qh=0 divisions land by step 35 (2 phases + finisher
                # drain), so the first 8 out-proj chunks interleave into the
                # qh=1 phases, one per 3 steps, at the step TOP: the scores
                # pool slot it takes then has a full step of slack before
                # the next scores allocation needs it
                if s == 16:  # PE observes wo2's DMA well before out-proj
                    nc.tensor.ldweights(wo2_sb[:, 0, 0:128])
                if s >= 38 and (s - 38) % 3 == 0 and next_mq[0] < 8:
                    emit_outproj(next_mq[0])
                    next_mq[0] += 1
                if s < 8:  # drip-feed remaining projection work
                    for _ in range(2):
                        if v_backlog:
                            v_backlog.pop(0)()
                        if proj_backlog:
                            proj_backlog.pop(0)()
                # The pair's scores MMs are INTERLEAVED (A,B,A,B): heads
                # A/B occupy disjoint PE row-halves (tile_position auto
                # (0,0)/(64,0)), so each MM's weight load hoists over the
                # other head's in-flight MM instead of serializing
                et = expp.tile([128, T], f16)
                if p == 0:
                    knt, koff = (g * 128) // QW, (g * 128) % QW
                spts = [
                    ps_sc.tile([128, HW2], f32, tag="spt", name=f"spt_{s}_{_h}")
                    for _h in range(2)
                ]
                for qs in range(NQS):
                    for h2 in range(2):
                        hb = h2 * 64
                        if p == 0:
                            lhsT = kTn[knt][hb : hb + 64, koff : koff + 128]
                            rhs = qT0h[qh][
                                hb : hb + 64, qs * QW2 : (qs + 1) * QW2
                            ]
                        else:
                            lhsT = kT1[hb : hb + 64, ts(g, 128)]
                            rhs = qT1[
                                hb : hb + 64,
                                qh * HW2 + qs * QW2 : qh * HW2 + (qs + 1) * QW2,
                            ]
                        nc.tensor.matmul(
                            spts[h2][:, qs * QW2 : (qs + 1) * QW2],
                            lhsT,
                            rhs,
                            start=True,
                            stop=True,
                        )
                for h2 in range(2):
                    nc.scalar.activation(
                        out=et[:, h2 * HW2 : (h2 + 1) * HW2],
                        in_=spts[h2][:],
                        func=mybir.ActivationFunctionType.Exp,
                        bias=exp_bias[:],
                        scale=1.0 / 8.0,
                    )
                expT[s] = et
                # AV matmuls AFTER the step's scores: the exp->next-scores
                # PSUM ping-pong then overlaps the AV block instead of
                # serializing behind it (scores(s+1) needs exp(s) retired;
                # placing AV between them absorbs the exp latency)
                for aq in av_sched.pop(s, []):
                    av_step(aq)
                # division finishers at the END of the step: their broadcast
                # matmul waits on a DVE reciprocal, and at the head of the
                # step it would stall the PE stream ahead of independent
                # scores/AV work (measured 2.6us ACT gaps per phase boundary)
                for _ in range(2):
                    if pending_div:
                        finish_div(*pending_div.pop(0))
                if s == 8:
                    # anything not drip-fed (shouldn't happen at T=2048)
                    while v_backlog:
                        v_backlog.pop(0)()
                    while proj_backlog:
                        proj_backlog.pop(0)()
            for s in sorted(av_sched):
                for _ in range(2):
                    if pending_div:
                        finish_div(*pending_div.pop(0))
                for aq in av_sched[s]:
                    av_step(aq)
            av_sched.clear()

            # ---- tail: remaining divisions + second-half out-projection ----
            # finishers FIRST: a chunk emitted before a finisher it reads
            # would cycle through the in-order DVE stream (deadlock)
            while pending_div:
                finish_div(*pending_div.pop(0))
            for mq in range(next_mq[0], T // 128):
                emit_outproj(
                    mq,
                    dma_eng=nc.scalar if mq % 2 else nc.sync,
                    act_copy=(mq % 2 == 1),
                )

    nc.finalize()
    return nc


def make_in_maps(x, W_attn, b_attn, W_out):
    """Shard full inputs across 8 cores: core c = (batch c//2, head-half c%2).

    Everything is pre-swizzled into partition-major layouts so device DMA
    descriptors are 2-4KB contiguous per partition:
      xT  [p, chunk, ko, t]  (channel = ko*128 + p, token = chunk*512 + t)
      w*  [p, ko, m]         (input channel = ko*128 + p)
      b*  [p, o]             (channel = o*128 + p)
      wo  [p, pr, n]         (y channel = pr*128 + p)
    """
    bf = ml_dtypes.bfloat16
    in_maps = []
    for c in range(N_CORES):
        b, hh = divmod(c, 2)
        sl = slice(hh * HPC * DH, (hh + 1) * HPC * DH)  # channel slice (256)

        def wsw(w):  # [512, 256] -> [128, 4, 256]
            return np.ascontiguousarray(
                w.reshape(4, 128, HPC * DH).transpose(1, 0, 2)
            ).astype(bf)

        xt = x[b].T  # [C, T]
        in_maps.append(
            {
                "xT": np.ascontiguousarray(
                    xt.reshape(4, 128, 4, 512).transpose(1, 2, 0, 3)
                ).astype(bf),
                "wq": wsw(W_attn[:, 0 * C :][:, sl]),
                "wk": wsw(W_attn[:, 1 * C :][:, sl]),
                "wv": wsw(W_attn[:, 2 * C :][:, sl]),
                "bq": np.ascontiguousarray(
                    b_attn[0 * C :][sl].reshape(2, 128).T, dtype=np.float32
                ),
                "bk": np.ascontiguousarray(
                    b_attn[1 * C :][sl].reshape(2, 128).T, dtype=np.float32
                ),
                "bv": np.ascontiguousarray(b_attn[2 * C :][sl], dtype=np.float32),
                "wo": np.ascontiguousarray(
                    W_out[sl, :].reshape(2, 128, C).transpose(1, 0, 2)
                ).astype(bf),
            }
        )
    return in_maps


def kernel(x, W_attn, b_attn, W_out, b_out, _trace=False):
    from concourse.bass_utils import run_bass_kernel_spmd

    x = np.asarray(x, dtype=np.float32)
    W_attn = np.asarray(W_attn, dtype=np.float32)
    b_attn = np.asarray(b_attn, dtype=np.float32)
    W_out = np.asarray(W_out, dtype=np.float32)
    b_out = np.asarray(b_out, dtype=np.float32)

    key = T_FULL
    if key not in _prog_cache:
        _prog_cache[key] = build_nc(T_FULL)
    nc = _prog_cache[key]

    in_maps = make_in_maps(x, W_attn, b_attn, W_out)
    res = run_bass_kernel_spmd(nc, in_maps, list(range(N_CORES)), trace=_trace)

    out = np.empty((B, T_FULL, C), dtype=np.float32)
    for b in range(B):
        out[b] = (
            res.results[2 * b]["out"].astype(np.float32)
            + res.results[2 * b + 1]["out"].astype(np.float32)
            + b_out
        )
    if _trace:
        kernel.last_exec_time_ns = res.exec_time_ns
        kernel.last_results = res
    return out



# revision 38
# speedup vs baseline: 1.0630x; 1.0050x over previous
"""Fused multi-head attention kernel for Trainium2, SPMD over 8 NeuronCores.

Problem: nn_MultiHeadAttention (B=4, T=2048, C=512, H=8 heads, Dh=64).
  qkv = x @ W_attn + b_attn ; split q,k,v ; per-head softmax(q k^T / 8) v ;
  out = y @ W_out + b_out

Sharding: core c handles batch b = c//2 and heads hh*4..hh*4+3 (hh = c%2).
Each core computes a partial out-projection over its 4 heads' channels;
the host sums the two partials per batch and adds b_out.

Device-side layout is fully "transposed" (token axis on the free dim):
  xT [C, T] -> qT,kT [64h, T] (per head on partitions 0..63/64..127),
  v in natural [T, 256] layout padded with a ones column per head,
  scoresT [kpos, qpos] tiles -> exp on ScalarE -> AV matmuls give
  yT [d, qpos] with an extra row = softmax denominator (ones-column trick).
Softmax skips max-subtraction: scores ~ N(0,1), |s|max < ~10, safe in fp32.
Matmul inputs are bf16 (PSUM accumulation fp32); exp input fp32 from PSUM.
"""

import sys

if "/opt/trn_rl_repo" not in sys.path:
    sys.path.insert(0, "/opt/trn_rl_repo")

import numpy as np
import ml_dtypes

B, T_FULL, C = 4, 2048, 512
H, DH = 8, 64
HPC = 4  # heads per core
N_CORES = 8

_prog_cache = {}


def build_nc(T=T_FULL):
    import concourse.bass as bass
    import concourse.tile as tile
    from concourse import bacc, mybir
    from concourse.bass import ts

    f32 = mybir.dt.float32
    bf16 = mybir.dt.bfloat16
    # attention-probability dtype: fp16 (11-bit mantissa) is ~16x more precise
    # than bf16 for exp outputs, same 1 cyc/row PE rate; exp(s/8 - 2) keeps the
    # largest value ~e^6 even for outlier scores, far from fp16's 65504 max.
    f16 = mybir.dt.float16
    EXP_SHIFT = -2.0

    KT = T // 128         # kpos chunks
    NQ = max(1, T // 512) # q tiles of 512
    QW = min(T, 512)      # q tile width
    CH = HPC * DH         # 256 channels per core per q/k/v

    def pbcast(ap, nparts):
        """Partition-broadcast a 1-D (free-only) AP to [nparts, ...] for DMA."""
        return bass.AP(
            tensor=ap.tensor, offset=ap.offset, ap=[[0, nparts]] + list(ap.ap)
        )

    def pbcast2(ap, nparts):
        """Same for a [1, N] AP: replace the partition dim with a stride-0
        broadcast so a DMA can replicate one SBUF row across partitions."""
        return bass.AP(
            tensor=ap.tensor, offset=ap.offset, ap=[[0, nparts]] + list(ap.ap)[1:]
        )

    # Bacc (not raw Bass): its finalize() runs move_matmul_waits_to_ldweights +
    # generate_event_semaphores, legalizing the TRN2 1-wait-per-instruction limit.
    nc = bacc.Bacc("TRN2")

    # all inputs are pre-swizzled on the host into partition-major layouts so
    # every DMA descriptor is 2-4KB contiguous (512B descriptors measured
    # ~5x under DMA-ring peak and 1.2-2.5us of descgen per instruction)
    xT = nc.dram_tensor("xT", [128, 4, 4, QW], bf16, kind="ExternalInput")
    wq = nc.dram_tensor("wq", [128, 4, CH], bf16, kind="ExternalInput")
    wk = nc.dram_tensor("wk", [128, 4, CH], bf16, kind="ExternalInput")
    wv = nc.dram_tensor("wv", [128, 4, CH], bf16, kind="ExternalInput")
    bq = nc.dram_tensor("bq", [128, 2], f32, kind="ExternalInput")
    bk = nc.dram_tensor("bk", [128, 2], f32, kind="ExternalInput")
    bv = nc.dram_tensor("bv", [CH], f32, kind="ExternalInput")
    wo = nc.dram_tensor("wo", [128, 2, C], bf16, kind="ExternalInput")
    # partial (per-core) contribution; host sums core pairs in f32, so bf16
    # is plenty and halves the writeback bytes
    out = nc.dram_tensor("out", [T, C], bf16, kind="ExternalOutput")

    with tile.TileContext(nc) as tc:
        with (
            tc.tile_pool(name="consts", bufs=1) as consts,
            tc.tile_pool(name="ps_sc", bufs=2, space="PSUM") as ps_sc,
            tc.tile_pool(name="ps512", bufs=4, space="PSUM") as ps512,
            tc.tile_pool(name="expp", bufs=10) as expp,
            tc.tile_pool(name="rsm", bufs=4) as rsm,
            tc.tile_pool(name="rbp", bufs=4) as rbp,
            tc.tile_pool(name="outp", bufs=3) as outp,
        ):
            # ---- constant loads ----
            # Two HWDGE queues (SP="sync", Activation="scalar") descgen in
            # parallel; DMAs are ordered by first-use so the first projection
            # chains start ~2us in instead of waiting for the whole 3MB.
            exp_bias = consts.tile([128, 1], f32)
            nc.vector.memset(exp_bias[:], EXP_SHIFT)
            # warmup activation: forces the ~2.7us ACT table load to run at
            # t=0, before the scalar-queue DMA descgens and first real exp
            warm = consts.tile([128, 1], f32)
            nc.scalar.activation(
                out=warm[:],
                in_=exp_bias[:],
                func=mybir.ActivationFunctionType.Exp,
            )
            # sync queue carries the compute-critical loads in need order;
            # the scalar queue takes the rest
            wk_sb = consts.tile([128, 4, CH], bf16)
            nc.sync.dma_start(wk_sb[:], wk[:])
            # xT in 4 token chunks: chain nt / v-chunk g only waits on its own
            # chunk's DMA (tile framework tracks region-level overlap)
            xT_sb = consts.tile([128, 4, 4, QW], bf16)
            nc.sync.dma_start(xT_sb[:, 0], xT[:, 0])
            wq_sb = consts.tile([128, 4, CH], bf16)
            nc.sync.dma_start(wq_sb[:], wq[:])
            for _c in range(1, 4):
                nc.sync.dma_start(xT_sb[:, _c], xT[:, _c])
            bq_sb = consts.tile([128, 2], f32)
            nc.scalar.dma_start(bq_sb[:], bq[:])
            bk_sb = consts.tile([128, 2], f32)
            nc.scalar.dma_start(bk_sb[:], bk[:])
            bv_sb = consts.tile([128, CH], f32)
            nc.scalar.dma_start(bv_sb[:], pbcast(bv[:], 128))
            wv_sb = consts.tile([128, 4, CH], bf16)
            nc.scalar.dma_start(wv_sb[:], wv[:])
            # head-PAIR rows: wo2_sb[:, pr, :] = W_out rows for heads 2pr,
            # 2pr+1 (channel = h*64+d), matching the packed y_pair layout so
            # the out-proj contracts K=128 (2 heads) per matmul
            wo2_sb = consts.tile([128, 2, C], bf16)
            nc.scalar.dma_start(wo2_sb[:], wo[:])
            # Pre-touch DMA-loaded tiles on DVE: tensor_scalar/tensor_tensor
            # instructions have too few sync-wait slots to wait on both a PE
            # semaphore and a DMA semaphore; a cheap DVE read here makes the
            # DVE clock observe the DMA completion so later ops need only the
            # PE wait (walrus NCC_INLA001 "Too many sync wait commands").
            touch = consts.tile([128, 8], f32)
            nc.vector.tensor_copy(out=touch[:, 0:2], in_=bq_sb[:])
            nc.vector.tensor_copy(out=touch[:, 2:4], in_=bk_sb[:])
            nc.vector.tensor_copy(out=touch[:, 4:5], in_=bv_sb[:, 0:1])
            # Same trick for the PE clock: a dummy ldweights per DMA-loaded
            # matmul input makes PE observe the DMA queues once, so real
            # matmuls never carry a DMA wait on top of their compute waits.
            # The PE queue is in-order, so these are STAGGERED by DMA arrival
            # (first-needed first); late arrivals (wv, xT chunks 2-3, wo2)
            # are touched from the drip-feed backlog / main loop instead of
            # head-blocking the first projection chains here.
            nc.tensor.ldweights(wk_sb[:, 0, 0:128])
            nc.tensor.ldweights(wq_sb[:, 0, 0:128])
            nc.tensor.ldweights(xT_sb[:, 0, 0, 0:128])
            # ones row for the K=1 broadcast matmul in the softmax division
            ones64 = consts.tile([1, DH], f16)
            nc.vector.memset(ones64[:], 1.0)

            # ---- computed tensors ----
            # group 0 (heads 0/1) is split into column tiles so the first
            # scores matmul only waits on 3 projection chains, not 8:
            # qT0h[qh] covers the qpos half a phase reads; kTn[nt] one 512 col
            # chunk of kT. Group 1 stays monolithic (it is drip-fed early).
            HW2_ = T // 2
            qT0h = [
                consts.tile([128, HW2_], bf16, tag=f"qT0h{i}", name=f"qT0h{i}")
                for i in range(2)
            ]
            kTn = [
                consts.tile([128, QW], bf16, tag=f"kTn{i}", name=f"kTn{i}")
                for i in range(NQ)
            ]
            qT1 = consts.tile([128, T], bf16, tag="qT1", name="qT1")
            kT1 = consts.tile([128, T], bf16, tag="kT1", name="kT1")
            # v (natural layout) padded with ones column: [128, KT, HPC, 65]
            v_ones = consts.tile([128, KT, HPC, DH + 1], f16)
            nc.vector.memset(v_ones[:, :, :, DH : DH + 1], 1.0)
            # yT packed per head-PAIR (head 2p at rows 0-63, 2p+1 at 64-127)
            # so out-proj matmuls contract K=128; odd heads' divisions write a
            # base-0 staging tile (DVE can't shift partitions) that a small
            # SBUF->SBUF DMA moves into the pair tile's high half
            y_pair = {
                (p, qh): consts.tile(
                    [128, T // 2], bf16, tag=f"yp{p}_{qh}", name=f"yp{p}_{qh}"
                )
                for p in range(2)
                for qh in range(2)
            }
            yTodd = {
                (h, qh): consts.tile(
                    [64, T // 2], bf16, tag=f"yo{h}_{qh}", name=f"yo{h}_{qh}"
                )
                for h in (1, 3)
                for qh in range(2)
            }

            # ---- Phase 1: QKV projection ----
            def qk_chain(m, w_sb, b_sb, dst_ap, nt):
                pt = ps512.tile([128, 512], f32, tag="mm512", name=f"qk_{m}_{nt}")
                for kt in range(4):
                    nc.tensor.matmul(
                        pt[:, :QW],
                        w_sb[:, kt, m * 128 : (m + 1) * 128],
                        xT_sb[:, nt, kt, :QW],
                        start=(kt == 0),
                        stop=(kt == 3),
                    )
                nc.vector.tensor_scalar_add(
                    out=dst_ap,
                    in0=pt[:, :QW],
                    scalar1=b_sb[:, m : m + 1],
                )

            def q0_dst(nt):
                qh = (nt * QW) // HW2_
                off = (nt * QW) % HW2_
                return qT0h[qh][:, off : off + QW]

            def v_chunk(g):
                pt = ps512.tile([128, 512], f32, tag="mm512", name=f"v_{g}")
                for kt in range(4):
                    nc.tensor.matmul(
                        pt[:, :CH],
                        xT_sb[:, g // 4, kt, (g % 4) * 128 : (g % 4) * 128 + 128],
                        wv_sb[:, kt, :],
                        start=(kt == 0),
                        stop=(kt == 3),
                    )
                nc.vector.tensor_add(
                    out=v_ones[:, g, :, 0:DH],
                    in0=pt[:, :CH].rearrange("p (h d) -> p h d", h=HPC),
                    in1=bv_sb[:].rearrange("p (h d) -> p h d", h=HPC),
                )

            # Pre-loop: only what scores step 0 needs — kT chunk 0 and the
            # first qpos-half of qT, both for group 0. Everything else is
            # drip-fed into the early loop steps (2 chains + 2 v-chunks per
            # step) while the AV PSUM-slot demand is still zero.
            nq_half = max(1, HW2_ // QW)  # q chains per qh half
            qk_chain(0, wk_sb, bk_sb, kTn[0][:, :QW], 0)
            for nt in range(nq_half):
                if nt > 0:  # PE observes xT chunk nt's DMA before using it
                    nc.tensor.ldweights(xT_sb[:, nt, 0, 0:128])
                qk_chain(0, wq_sb, bq_sb, q0_dst(nt), nt)
            # late-arriving DMAs (xT chunks 2-3, wv land ~5-9us in) are
            # observed by a dummy ldweights placed at the LAST moment before
            # their first user, so the in-order PE stream never head-blocks
            proj_backlog = []
            for nt in range(1, NQ):  # k chunks in need-order (g = 4*nt)
                if nt >= 2:
                    proj_backlog.append(
                        lambda nt=nt: nc.tensor.ldweights(
                            xT_sb[:, nt, 0, 0:128]
                        )
                    )
                proj_backlog.append(
                    lambda nt=nt: qk_chain(0, wk_sb, bk_sb, kTn[nt][:, :QW], nt)
                )
            for nt in range(nq_half, NQ):
                proj_backlog.append(
                    lambda nt=nt: qk_chain(0, wq_sb, bq_sb, q0_dst(nt), nt)
                )
            for nt in range(NQ):
                proj_backlog.append(
                    lambda nt=nt: qk_chain(1, wq_sb, bq_sb, qT1[:, ts(nt, QW)], nt)
                )
                proj_backlog.append(
                    lambda nt=nt: qk_chain(1, wk_sb, bk_sb, kT1[:, ts(nt, QW)], nt)
                )
            # wv lands ~4us in, before the drip loop's first v_chunk; touch it
            # here (not in the backlog — the backlog MUST drain by step 8,
            # when the AV accumulators take every ps512 slot)
            nc.tensor.ldweights(wv_sb[:, 0, 0:128])
            v_backlog = [lambda g=g: v_chunk(g) for g in range(KT)]

            # ---- Phase 2: attention ----
            # Head-PAIR processing: heads hA=2p (partitions 0-63) and hB=2p+1
            # (64-127) issue adjacent row-tiled matmuls that run concurrently
            # on the PE array, writing disjoint column ranges of one scores
            # PSUM tile [128, T]: cols [0, T/2) = hA's qpos half, [T/2, T) =
            # hB's same qpos half. Each pair is covered in 2 "qh" phases.
            # AV matmuls lag 2 steps behind scores/exp (lag-2 pipeline).
            # bank-disjointness of the concurrent head-pair matmuls requires
            # each head's column range to cover whole PSUM banks (>=512 f32)
            assert T >= 1024, "pair-packed scores need T/2 >= 512 (PSUM bank)"
            HW2 = T // 2              # qpos width per head per scores tile
            QW2 = min(512, HW2)       # AV / division chunk width
            NQS = HW2 // QW2          # AV chains per head per phase
            expT = {}                 # step -> sbuf tile [128, T]
            av_ps = {}                # (h, qs) -> psum tile
            pending_div = []          # deferred division finishers
            next_mq = [0]             # out-proj chunks emitted in-loop
            NPH = 2 * 2               # pairs * qh phases
            NSTEP = NPH * KT

            # Phase order (p,qh): (0,0) (1,0) (0,1) (1,1) — both pairs finish
            # qh=0 by mid-kernel, so the first half of the out-projection (and
            # its DMA writeback) overlaps the qh=1 attention phases.
            def decode(s):
                ph, g = divmod(s, KT)
                qh, p = divmod(ph, 2)
                return p, qh, g

            # AV-step retiming. Phase 0 delays AV to step 8 (catch-up 2/step)
            # so the early steps' PSUM slots are free for the interleaved
            # v-projection / qk group-1 chains; later phases delay their first
            # AV allocations to +6 so the previous phase's division finishers
            # (all popped by +3) have released every AV slot — allocating
            # earlier would let a PE matmul wait on a slot whose release is
            # behind it in the PE stream (deadlock).
            av_sched = {}
            for _ph in range(NPH):
                for _g in range(KT):
                    _aq = _ph * KT + _g
                    if _ph == 0:
                        _run = max(_g + 2, 8 + _g // 2)
                    else:
                        _run = _ph * KT + max(_g + 2, 4 + _g // 2)
                    av_sched.setdefault(_run, []).append(_aq)

            def av_step(s):
                p, qh, g = decode(s)
                for h2 in range(2):
                    h = 2 * p + h2
                    for qs in range(NQS):
                        if g == 0:
                            av_ps[(h, qs)] = ps512.tile(
                                [128, 512], f32, tag="mm512", name=f"av_{s}_{h2}_{qs}"
                            )
                        nc.tensor.matmul(
                            av_ps[(h, qs)][: DH + 1, :QW2],
                            v_ones[:, g, h, :],
                            expT[s][:, h2 * HW2 + qs * QW2 : h2 * HW2 + (qs + 1) * QW2],
                            start=(g == 0),
                            stop=(g == KT - 1),
                        )
                if g == KT - 1:
                    emit_divs(p, qh)

            def emit_divs(p, qh):
                # Reciprocals now (DVE, off critical path), then a DMA
                # partition-broadcast replicates 1/denom across 64 partitions
                # (DMA rings are idle mid-attention; the old K=1 PE broadcast
                # + DVE copy cost ~430ns PE + ~620ns DVE per chain)
                for h2 in range(2):
                    h = 2 * p + h2
                    for qs in range(NQS):
                        av = av_ps.pop((h, qs))
                        # custom-DVE ops corrupt data when reading PSUM
                        # directly — bounce the denominator row to SBUF
                        den = rsm.tile([1, 512], f32, tag="den", name=f"dn_{h}_{qh}_{qs}")
                        nc.vector.tensor_copy(
                            out=den[:, :QW2], in_=av[DH : DH + 1, :QW2]
                        )
                        rf = rsm.tile([1, 512], f32, tag="rf", name=f"rf_{h}_{qh}_{qs}")
                        nc.vector.reciprocal_approx_fast(
                            out=rf[:, :QW2], in_=den[:, :QW2]
                        )
                        rb = rbp.tile([64, 512], f32)
                        # replicate across partitions on the (idle) GpSimd
                        # engine: SBUF APs can't stride-0 the partition dim,
                        # so a DMA can't do this and the PE K=1 matmul
                        # broadcast cost ~430ns PE + ~620ns DVE per chain
                        nc.gpsimd.partition_broadcast(rb[:, :QW2], rf[0:1, :QW2])
                        pending_div.append((h, qh, qs, rb, av))

            def finish_div(h, qh, qs, rb, av):
                # a 1-element DVE touch observes the broadcast-DMA completion
                # so the multiply itself needs only the PE wait
                nc.vector.tensor_copy(out=touch[0:1, 5:6], in_=rb[0:1, 0:1])
                col = qs * QW2
                if h % 2 == 0:
                    dst = y_pair[(h // 2, qh)][0:64, col : col + QW2]
                else:
                    dst = yTodd[(h, qh)][:, col : col + QW2]
                nc.vector.tensor_mul(
                    out=dst,
                    in0=av[0:DH, :QW2],
                    in1=rb[:, :QW2],
                )
                if h % 2 == 1:
                    # move the odd head's slice into the pair tile's high half
                    nc.sync.dma_start(
                        y_pair[(h // 2, qh)][64:128, col : col + QW2],
                        yTodd[(h, qh)][:, col : col + QW2],
                    )

            # ---- out-projection chunk emitter (partial, this core's heads) ----
            # PSUM comes from the scores pool (same tag = same slots); early
            # chunks are interleaved into the qh=1 attention phases.
            def emit_outproj(mq, dma_eng=None, act_copy=False):
                opt = ps_sc.tile([128, HW2], f32, tag="spt", name=f"op_{mq}")
                mqh, mcol = (mq * 128) // HW2, (mq * 128) % HW2
                for pp in range(2):
                    nc.tensor.matmul(
                        opt[:, :512],
                        y_pair[(pp, mqh)][:, mcol : mcol + 128],
                        wo2_sb[:, pp, :],
                        start=(pp == 0),
                        stop=(pp == 1),
                    )
                ot = outp.tile([128, 512], bf16)
                # tail chunks copy on the (then-idle) ACT engine so the DVE
                # doesn't pace the PSUM slot recycling; in-loop chunks use
                # DVE (ACT is the bottleneck mid-attention)
                if act_copy:
                    nc.scalar.copy(out=ot[:], in_=opt[:, :512])
                else:
                    nc.vector.tensor_copy(out=ot[:], in_=opt[:, :512])
                (dma_eng or nc.sync).dma_start(out[ts(mq, 128), :], ot[:])

            # Two scores PSUM tiles per step (one per head of the pair, 2
            # banks each) from a bufs=2 pool: head-A's scores of step s+1
            # only wait on exp-A(s) — exp and scores ping-pong with full
            # ACT overlap instead of serializing on one tile.
            for s in range(NSTEP):
                p, qh, g = decode(s)
                # all qh=0 divisions land by step 35 (2 phases + finisher
                # drain), so the first 8 out-proj chunks interleave into the
                # qh=1 phases, one per 3 steps, at the step TOP: the scores
                # pool slot it takes then has a full step of slack before
                # the next scores allocation needs it
                if s == 16:  # PE observes wo2's DMA well before out-proj
                    nc.tensor.ldweights(wo2_sb[:, 0, 0:128])
                if s >= 38 and (s - 38) % 3 == 0 and next_mq[0] < 8:
                    emit_outproj(next_mq[0])
                    next_mq[0] += 1
                if s < 8:  # drip-feed remaining projection work
                    for _ in range(2):
                        if v_backlog:
                            v_backlog.pop(0)()
                        if proj_backlog:
                            proj_backlog.pop(0)()
                # The pair's scores MMs are INTERLEAVED (A,B,A,B): heads
                # A/B occupy disjoint PE row-halves (tile_position auto
                # (0,0)/(64,0)), so each MM's weight load hoists over the
                # other head's in-flight MM instead of serializing
                et = expp.tile([128, T], f16)
                if p == 0:
                    knt, koff = (g * 128) // QW, (g * 128) % QW
                spts = [
                    ps_sc.tile([128, HW2], f32, tag="spt", name=f"spt_{s}_{_h}")
                    for _h in range(2)
                ]
                for qs in range(NQS):
                    for h2 in range(2):
                        hb = h2 * 64
                        if p == 0:
                            lhsT = kTn[knt][hb : hb + 64, koff : koff + 128]
                            rhs = qT0h[qh][
                                hb : hb + 64, qs * QW2 : (qs + 1) * QW2
                            ]
                        else:
                            lhsT = kT1[hb : hb + 64, ts(g, 128)]
                            rhs = qT1[
                                hb : hb + 64,
                                qh * HW2 + qs * QW2 : qh * HW2 + (qs + 1) * QW2,
                            ]
                        nc.tensor.matmul(
                            spts[h2][:, qs * QW2 : (qs + 1) * QW2],
                            lhsT,
                            rhs,
                            start=True,
                            stop=True,
                        )
                for h2 in range(2):
                    nc.scalar.activation(
                        out=et[:, h2 * HW2 : (h2 + 1) * HW2],
                        in_=spts[h2][:],
                        func=mybir.ActivationFunctionType.Exp,
                        bias=exp_bias[:],
                        scale=1.0 / 8.0,
                    )
                expT[s] = et
                # Phase-start steps (no AVs yet, catch-up delayed) leave PE
                # micro-idle gaps that trip the HAM clock-gate back to K=4/8
                # (measured 10-75us throttle latches starting exactly at the
                # phase-0/1 boundary). A dummy ldweights that WAITS on this
                # step's exp executes mid-gap and keeps the activity monitor
                # fed. ~107ns each, only on the 4 sparse steps per phase.
                if s >= 16 and s % KT < 4:
                    nc.tensor.ldweights(et[:, 0:128])
                    nc.tensor.ldweights(et[:, HW2 : HW2 + 128])
                # AV matmuls AFTER the step's scores: the exp->next-scores
                # PSUM ping-pong then overlaps the AV block instead of
                # serializing behind it (scores(s+1) needs exp(s) retired;
                # placing AV between them absorbs the exp latency)
                for aq in av_sched.pop(s, []):
                    av_step(aq)
                # division finishers at the END of the step: their broadcast
                # matmul waits on a DVE reciprocal, and at the head of the
                # step it would stall the PE stream ahead of independent
                # scores/AV work (measured 2.6us ACT gaps per phase boundary)
                for _ in range(2):
                    if pending_div:
                        finish_div(*pending_div.pop(0))
                if s == 8:
                    # anything not drip-fed (shouldn't happen at T=2048)
                    while v_backlog:
                        v_backlog.pop(0)()
                    while proj_backlog:
                        proj_backlog.pop(0)()
            for s in sorted(av_sched):
                for _ in range(2):
                    if pending_div:
                        finish_div(*pending_div.pop(0))
                for aq in av_sched[s]:
                    av_step(aq)
            av_sched.clear()

            # ---- tail: remaining divisions + second-half out-projection ----
            # finishers FIRST: a chunk emitted before a finisher it reads
            # would cycle through the in-order DVE stream (deadlock)
            while pending_div:
                finish_div(*pending_div.pop(0))
            for mq in range(next_mq[0], T // 128):
                emit_outproj(
                    mq,
                    dma_eng=nc.scalar if mq % 2 else nc.sync,
                    act_copy=(mq % 2 == 1),
                )

    nc.finalize()
    return nc


def make_in_maps(x, W_attn, b_attn, W_out):
    """Shard full inputs across 8 cores: core c = (batch c//2, head-half c%2).

    Everything is pre-swizzled into partition-major layouts so device DMA
    descriptors are 2-4KB contiguous per partition:
      xT  [p, chunk, ko, t]  (channel = ko*128 + p, token = chunk*512 + t)
      w*  [p, ko, m]         (input channel = ko*128 + p)
      b*  [p, o]             (channel = o*128 + p)
      wo  [p, pr, n]         (y channel = pr*128 + p)
    """
    bf = ml_dtypes.bfloat16
    in_maps = []
    for c in range(N_CORES):
        b, hh = divmod(c, 2)
        sl = slice(hh * HPC * DH, (hh + 1) * HPC * DH)  # channel slice (256)

        def wsw(w):  # [512, 256] -> [128, 4, 256]
            return np.ascontiguousarray(
                w.reshape(4, 128, HPC * DH).transpose(1, 0, 2)
            ).astype(bf)

        xt = x[b].T  # [C, T]
        in_maps.append(
            {
                "xT": np.ascontiguousarray(
                    xt.reshape(4, 128, 4, 512).transpose(1, 2, 0, 3)
                ).astype(bf),
                "wq": wsw(W_attn[:, 0 * C :][:, sl]),
                "wk": wsw(W_attn[:, 1 * C :][:, sl]),
                "wv": wsw(W_attn[:, 2 * C :][:, sl]),
                "bq": np.ascontiguousarray(
                    b_attn[0 * C :][sl].reshape(2, 128).T, dtype=np.float32
                ),
                "bk": np.ascontiguousarray(
                    b_attn[1 * C :][sl].reshape(2, 128).T, dtype=np.float32
                ),
                "bv": np.ascontiguousarray(b_attn[2 * C :][sl], dtype=np.float32),
                "wo": np.ascontiguousarray(
                    W_out[sl, :].reshape(2, 128, C).transpose(1, 0, 2)
                ).astype(bf),
            }
        )
    return in_maps


def kernel(x, W_attn, b_attn, W_out, b_out, _trace=False):
    from concourse.bass_utils import run_bass_kernel_spmd

    x = np.asarray(x, dtype=np.float32)
    W_attn = np.asarray(W_attn, dtype=np.float32)
    b_attn = np.asarray(b_attn, dtype=np.float32)
    W_out = np.asarray(W_out, dtype=np.float32)
    b_out = np.asarray(b_out, dtype=np.float32)

    key = T_FULL
    if key not in _prog_cache:
        _prog_cache[key] = build_nc(T_FULL)
    nc = _prog_cache[key]

    in_maps = make_in_maps(x, W_attn, b_attn, W_out)
    res = run_bass_kernel_spmd(nc, in_maps, list(range(N_CORES)), trace=_trace)

    out = np.empty((B, T_FULL, C), dtype=np.float32)
    for b in range(B):
        out[b] = (
            res.results[2 * b]["out"].astype(np.float32)
            + res.results[2 * b + 1]["out"].astype(np.float32)
            + b_out
        )
    if _trace:
        kernel.last_exec_time_ns = res.exec_time_ns
        kernel.last_results = res
    return out



# revision 39
# speedup vs baseline: 1.1343x; 1.0671x over previous
"""Fused multi-head attention kernel for Trainium2, SPMD over 8 NeuronCores.

Problem: nn_MultiHeadAttention (B=4, T=2048, C=512, H=8 heads, Dh=64).
  qkv = x @ W_attn + b_attn ; split q,k,v ; per-head softmax(q k^T / 8) v ;
  out = y @ W_out + b_out

Sharding: core c handles batch b = c//2 and heads hh*4..hh*4+3 (hh = c%2).
Each core computes a partial out-projection over its 4 heads' channels;
the host sums the two partials per batch and adds b_out.

Device-side layout is fully "transposed" (token axis on the free dim):
  xT [C, T] -> qT,kT [64h, T] (per head on partitions 0..63/64..127),
  v in natural [T, 256] layout padded with a ones column per head,
  scoresT [kpos, qpos] tiles -> exp on ScalarE -> AV matmuls give
  yT [d, qpos] with an extra row = softmax denominator (ones-column trick).
Softmax skips max-subtraction: scores ~ N(0,1), |s|max < ~10, safe in fp32.
Matmul inputs are bf16 (PSUM accumulation fp32); exp input fp32 from PSUM.
"""

import sys

if "/opt/trn_rl_repo" not in sys.path:
    sys.path.insert(0, "/opt/trn_rl_repo")

import numpy as np
import ml_dtypes

B, T_FULL, C = 4, 2048, 512
H, DH = 8, 64
HPC = 4  # heads per core
N_CORES = 8

_prog_cache = {}


def build_nc(T=T_FULL):
    import concourse.bass as bass
    import concourse.tile as tile
    from concourse import bacc, mybir
    from concourse.bass import ts

    f32 = mybir.dt.float32
    bf16 = mybir.dt.bfloat16
    # attention-probability dtype: fp16 (11-bit mantissa) is ~16x more precise
    # than bf16 for exp outputs, same 1 cyc/row PE rate; exp(s/8 - 2) keeps the
    # largest value ~e^6 even for outlier scores, far from fp16's 65504 max.
    f16 = mybir.dt.float16
    EXP_SHIFT = -2.0

    KT = T // 128         # kpos chunks
    NQ = max(1, T // 512) # q tiles of 512
    QW = min(T, 512)      # q tile width
    CH = HPC * DH         # 256 channels per core per q/k/v

    def pbcast(ap, nparts):
        """Partition-broadcast a 1-D (free-only) AP to [nparts, ...] for DMA."""
        return bass.AP(
            tensor=ap.tensor, offset=ap.offset, ap=[[0, nparts]] + list(ap.ap)
        )

    def pbcast2(ap, nparts):
        """Same for a [1, N] AP: replace the partition dim with a stride-0
        broadcast so a DMA can replicate one SBUF row across partitions."""
        return bass.AP(
            tensor=ap.tensor, offset=ap.offset, ap=[[0, nparts]] + list(ap.ap)[1:]
        )

    # Bacc (not raw Bass): its finalize() runs move_matmul_waits_to_ldweights +
    # generate_event_semaphores, legalizing the TRN2 1-wait-per-instruction limit.
    nc = bacc.Bacc("TRN2")

    # all inputs are pre-swizzled on the host into partition-major layouts so
    # every DMA descriptor is 2-4KB contiguous (512B descriptors measured
    # ~5x under DMA-ring peak and 1.2-2.5us of descgen per instruction)
    xT = nc.dram_tensor("xT", [128, 4, 4, QW], bf16, kind="ExternalInput")
    wq = nc.dram_tensor("wq", [128, 4, CH], bf16, kind="ExternalInput")
    wk = nc.dram_tensor("wk", [128, 4, CH], bf16, kind="ExternalInput")
    wv = nc.dram_tensor("wv", [128, 4, CH], bf16, kind="ExternalInput")
    bq = nc.dram_tensor("bq", [128, 2], f32, kind="ExternalInput")
    bk = nc.dram_tensor("bk", [128, 2], f32, kind="ExternalInput")
    bv = nc.dram_tensor("bv", [CH], f32, kind="ExternalInput")
    wo = nc.dram_tensor("wo", [128, 2, C], bf16, kind="ExternalInput")
    # partial (per-core) contribution; host sums core pairs in f32, so bf16
    # is plenty and halves the writeback bytes
    out = nc.dram_tensor("out", [T, C], bf16, kind="ExternalOutput")

    with tile.TileContext(nc) as tc:
        with (
            tc.tile_pool(name="consts", bufs=1) as consts,
            tc.tile_pool(name="ps_sc", bufs=2, space="PSUM") as ps_sc,
            tc.tile_pool(name="ps512", bufs=4, space="PSUM") as ps512,
            tc.tile_pool(name="expp", bufs=10) as expp,
            tc.tile_pool(name="rsm", bufs=4) as rsm,
            tc.tile_pool(name="rbp", bufs=4) as rbp,
            tc.tile_pool(name="outp", bufs=3) as outp,
        ):
            # ---- constant loads ----
            # Two HWDGE queues (SP="sync", Activation="scalar") descgen in
            # parallel; DMAs are ordered by first-use so the first projection
            # chains start ~2us in instead of waiting for the whole 3MB.
            exp_bias = consts.tile([128, 1], f32)
            nc.vector.memset(exp_bias[:], EXP_SHIFT)
            # warmup activation: forces the ~2.7us ACT table load to run at
            # t=0, before the scalar-queue DMA descgens and first real exp
            warm = consts.tile([128, 1], f32)
            nc.scalar.activation(
                out=warm[:],
                in_=exp_bias[:],
                func=mybir.ActivationFunctionType.Exp,
            )
            # sync queue carries the compute-critical loads in need order;
            # the scalar queue takes the rest
            wk_sb = consts.tile([128, 4, CH], bf16)
            nc.sync.dma_start(wk_sb[:], wk[:])
            # xT in 4 token chunks: chain nt / v-chunk g only waits on its own
            # chunk's DMA (tile framework tracks region-level overlap)
            xT_sb = consts.tile([128, 4, 4, QW], bf16)
            nc.sync.dma_start(xT_sb[:, 0], xT[:, 0])
            wq_sb = consts.tile([128, 4, CH], bf16)
            nc.sync.dma_start(wq_sb[:], wq[:])
            for _c in range(1, 4):
                nc.sync.dma_start(xT_sb[:, _c], xT[:, _c])
            bq_sb = consts.tile([128, 2], f32)
            nc.scalar.dma_start(bq_sb[:], bq[:])
            bk_sb = consts.tile([128, 2], f32)
            nc.scalar.dma_start(bk_sb[:], bk[:])
            bv_sb = consts.tile([128, CH], f32)
            nc.scalar.dma_start(bv_sb[:], pbcast(bv[:], 128))
            wv_sb = consts.tile([128, 4, CH], bf16)
            nc.scalar.dma_start(wv_sb[:], wv[:])
            # head-PAIR rows: wo2_sb[:, pr, :] = W_out rows for heads 2pr,
            # 2pr+1 (channel = h*64+d), matching the packed y_pair layout so
            # the out-proj contracts K=128 (2 heads) per matmul
            wo2_sb = consts.tile([128, 2, C], bf16)
            nc.scalar.dma_start(wo2_sb[:], wo[:])
            # PE clock warmup: the HAM gate only ramps 1.2->2.4GHz after
            # ~3.4us of sustained PE activity, and the first real work lands
            # at ~12.5us (DMA trigger latency); burn the warmup on dummy
            # matmuls during the DMA wait so the projection chains run at
            # full clock (measured 427ns vs 216ns per chain MM cold/warm)
            wrm = consts.tile([128, 128], bf16)
            nc.vector.memset(wrm[:], 0.0)
            wps = ps_sc.tile([128, T // 2], f32, tag="spt", name="warm_ps")
            for _i in range(36):
                nc.tensor.matmul(
                    wps[:, 0:128],
                    wrm[:],
                    wrm[:],
                    start=(_i == 0),
                    stop=(_i == 35),
                )
            # Pre-touch DMA-loaded tiles on DVE: tensor_scalar/tensor_tensor
            # instructions have too few sync-wait slots to wait on both a PE
            # semaphore and a DMA semaphore; a cheap DVE read here makes the
            # DVE clock observe the DMA completion so later ops need only the
            # PE wait (walrus NCC_INLA001 "Too many sync wait commands").
            touch = consts.tile([128, 8], f32)
            nc.vector.tensor_copy(out=touch[:, 0:2], in_=bq_sb[:])
            nc.vector.tensor_copy(out=touch[:, 2:4], in_=bk_sb[:])
            nc.vector.tensor_copy(out=touch[:, 4:5], in_=bv_sb[:, 0:1])
            # Same trick for the PE clock: a dummy ldweights per DMA-loaded
            # matmul input makes PE observe the DMA queues once, so real
            # matmuls never carry a DMA wait on top of their compute waits.
            # The PE queue is in-order, so these are STAGGERED by DMA arrival
            # (first-needed first); late arrivals (wv, xT chunks 2-3, wo2)
            # are touched from the drip-feed backlog / main loop instead of
            # head-blocking the first projection chains here.
            nc.tensor.ldweights(wk_sb[:, 0, 0:128])
            nc.tensor.ldweights(wq_sb[:, 0, 0:128])
            nc.tensor.ldweights(xT_sb[:, 0, 0, 0:128])
            # ones row for the K=1 broadcast matmul in the softmax division
            ones64 = consts.tile([1, DH], f16)
            nc.vector.memset(ones64[:], 1.0)

            # ---- computed tensors ----
            # group 0 (heads 0/1) is split into column tiles so the first
            # scores matmul only waits on 3 projection chains, not 8:
            # qT0h[qh] covers the qpos half a phase reads; kTn[nt] one 512 col
            # chunk of kT. Group 1 stays monolithic (it is drip-fed early).
            HW2_ = T // 2
            qT0h = [
                consts.tile([128, HW2_], bf16, tag=f"qT0h{i}", name=f"qT0h{i}")
                for i in range(2)
            ]
            kTn = [
                consts.tile([128, QW], bf16, tag=f"kTn{i}", name=f"kTn{i}")
                for i in range(NQ)
            ]
            qT1 = consts.tile([128, T], bf16, tag="qT1", name="qT1")
            kT1 = consts.tile([128, T], bf16, tag="kT1", name="kT1")
            # v (natural layout) padded with ones column: [128, KT, HPC, 65]
            v_ones = consts.tile([128, KT, HPC, DH + 1], f16)
            nc.vector.memset(v_ones[:, :, :, DH : DH + 1], 1.0)
            # yT packed per head-PAIR (head 2p at rows 0-63, 2p+1 at 64-127)
            # so out-proj matmuls contract K=128; odd heads' divisions write a
            # base-0 staging tile (DVE can't shift partitions) that a small
            # SBUF->SBUF DMA moves into the pair tile's high half
            y_pair = {
                (p, qh): consts.tile(
                    [128, T // 2], bf16, tag=f"yp{p}_{qh}", name=f"yp{p}_{qh}"
                )
                for p in range(2)
                for qh in range(2)
            }
            yTodd = {
                (h, qh): consts.tile(
                    [64, T // 2], bf16, tag=f"yo{h}_{qh}", name=f"yo{h}_{qh}"
                )
                for h in (1, 3)
                for qh in range(2)
            }

            # ---- Phase 1: QKV projection ----
            def qk_chain(m, w_sb, b_sb, dst_ap, nt):
                pt = ps512.tile([128, 512], f32, tag="mm512", name=f"qk_{m}_{nt}")
                for kt in range(4):
                    nc.tensor.matmul(
                        pt[:, :QW],
                        w_sb[:, kt, m * 128 : (m + 1) * 128],
                        xT_sb[:, nt, kt, :QW],
                        start=(kt == 0),
                        stop=(kt == 3),
                    )
                nc.vector.tensor_scalar_add(
                    out=dst_ap,
                    in0=pt[:, :QW],
                    scalar1=b_sb[:, m : m + 1],
                )

            def q0_dst(nt):
                qh = (nt * QW) // HW2_
                off = (nt * QW) % HW2_
                return qT0h[qh][:, off : off + QW]

            def v_chunk(g):
                pt = ps512.tile([128, 512], f32, tag="mm512", name=f"v_{g}")
                for kt in range(4):
                    nc.tensor.matmul(
                        pt[:, :CH],
                        xT_sb[:, g // 4, kt, (g % 4) * 128 : (g % 4) * 128 + 128],
                        wv_sb[:, kt, :],
                        start=(kt == 0),
                        stop=(kt == 3),
                    )
                nc.vector.tensor_add(
                    out=v_ones[:, g, :, 0:DH],
                    in0=pt[:, :CH].rearrange("p (h d) -> p h d", h=HPC),
                    in1=bv_sb[:].rearrange("p (h d) -> p h d", h=HPC),
                )

            # Pre-loop: only what scores step 0 needs — kT chunk 0 and the
            # first qpos-half of qT, both for group 0. Everything else is
            # drip-fed into the early loop steps (2 chains + 2 v-chunks per
            # step) while the AV PSUM-slot demand is still zero.
            nq_half = max(1, HW2_ // QW)  # q chains per qh half
            qk_chain(0, wk_sb, bk_sb, kTn[0][:, :QW], 0)
            for nt in range(nq_half):
                if nt > 0:  # PE observes xT chunk nt's DMA before using it
                    nc.tensor.ldweights(xT_sb[:, nt, 0, 0:128])
                qk_chain(0, wq_sb, bq_sb, q0_dst(nt), nt)
            # late-arriving DMAs (xT chunks 2-3, wv land ~5-9us in) are
            # observed by a dummy ldweights placed at the LAST moment before
            # their first user, so the in-order PE stream never head-blocks
            proj_backlog = []
            for nt in range(1, NQ):  # k chunks in need-order (g = 4*nt)
                if nt >= 2:
                    proj_backlog.append(
                        lambda nt=nt: nc.tensor.ldweights(
                            xT_sb[:, nt, 0, 0:128]
                        )
                    )
                proj_backlog.append(
                    lambda nt=nt: qk_chain(0, wk_sb, bk_sb, kTn[nt][:, :QW], nt)
                )
            for nt in range(nq_half, NQ):
                proj_backlog.append(
                    lambda nt=nt: qk_chain(0, wq_sb, bq_sb, q0_dst(nt), nt)
                )
            for nt in range(NQ):
                proj_backlog.append(
                    lambda nt=nt: qk_chain(1, wq_sb, bq_sb, qT1[:, ts(nt, QW)], nt)
                )
                proj_backlog.append(
                    lambda nt=nt: qk_chain(1, wk_sb, bk_sb, kT1[:, ts(nt, QW)], nt)
                )
            # wv lands ~4us in, before the drip loop's first v_chunk; touch it
            # here (not in the backlog — the backlog MUST drain by step 8,
            # when the AV accumulators take every ps512 slot)
            nc.tensor.ldweights(wv_sb[:, 0, 0:128])
            v_backlog = [lambda g=g: v_chunk(g) for g in range(KT)]

            # ---- Phase 2: attention ----
            # Head-PAIR processing: heads hA=2p (partitions 0-63) and hB=2p+1
            # (64-127) issue adjacent row-tiled matmuls that run concurrently
            # on the PE array, writing disjoint column ranges of one scores
            # PSUM tile [128, T]: cols [0, T/2) = hA's qpos half, [T/2, T) =
            # hB's same qpos half. Each pair is covered in 2 "qh" phases.
            # AV matmuls lag 2 steps behind scores/exp (lag-2 pipeline).
            # bank-disjointness of the concurrent head-pair matmuls requires
            # each head's column range to cover whole PSUM banks (>=512 f32)
            assert T >= 1024, "pair-packed scores need T/2 >= 512 (PSUM bank)"
            HW2 = T // 2              # qpos width per head per scores tile
            QW2 = min(512, HW2)       # AV / division chunk width
            NQS = HW2 // QW2          # AV chains per head per phase
            expT = {}                 # step -> sbuf tile [128, T]
            av_ps = {}                # (h, qs) -> psum tile
            pending_div = []          # deferred division finishers
            next_mq = [0]             # out-proj chunks emitted in-loop
            NPH = 2 * 2               # pairs * qh phases
            NSTEP = NPH * KT

            # Phase order (p,qh): (0,0) (1,0) (0,1) (1,1) — both pairs finish
            # qh=0 by mid-kernel, so the first half of the out-projection (and
            # its DMA writeback) overlaps the qh=1 attention phases.
            def decode(s):
                ph, g = divmod(s, KT)
                qh, p = divmod(ph, 2)
                return p, qh, g

            # AV-step retiming. Phase 0 delays AV to step 8 (catch-up 2/step)
            # so the early steps' PSUM slots are free for the interleaved
            # v-projection / qk group-1 chains; later phases delay their first
            # AV allocations to +6 so the previous phase's division finishers
            # (all popped by +3) have released every AV slot — allocating
            # earlier would let a PE matmul wait on a slot whose release is
            # behind it in the PE stream (deadlock).
            av_sched = {}
            for _ph in range(NPH):
                for _g in range(KT):
                    _aq = _ph * KT + _g
                    if _ph == 0:
                        _run = max(_g + 2, 8 + _g // 2)
                    else:
                        # +2 (was +4): division finishers now release AV
                        # slots via DVE/GpSimd ops popped at steps +0/+1, so
                        # the phase's AVs can resume 2 steps earlier, filling
                        # the sparse boundary steps that trip the HAM gate
                        _run = _ph * KT + max(_g + 2, 2 + _g // 2)
                    av_sched.setdefault(_run, []).append(_aq)

            def av_step(s):
                p, qh, g = decode(s)
                for h2 in range(2):
                    h = 2 * p + h2
                    for qs in range(NQS):
                        if g == 0:
                            av_ps[(h, qs)] = ps512.tile(
                                [128, 512], f32, tag="mm512", name=f"av_{s}_{h2}_{qs}"
                            )
                        nc.tensor.matmul(
                            av_ps[(h, qs)][: DH + 1, :QW2],
                            v_ones[:, g, h, :],
                            expT[s][:, h2 * HW2 + qs * QW2 : h2 * HW2 + (qs + 1) * QW2],
                            start=(g == 0),
                            stop=(g == KT - 1),
                        )
                if g == KT - 1:
                    emit_divs(p, qh)

            def emit_divs(p, qh):
                # Reciprocals now (DVE, off critical path), then a DMA
                # partition-broadcast replicates 1/denom across 64 partitions
                # (DMA rings are idle mid-attention; the old K=1 PE broadcast
                # + DVE copy cost ~430ns PE + ~620ns DVE per chain)
                for h2 in range(2):
                    h = 2 * p + h2
                    for qs in range(NQS):
                        av = av_ps.pop((h, qs))
                        # custom-DVE ops corrupt data when reading PSUM
                        # directly — bounce the denominator row to SBUF
                        den = rsm.tile([1, 512], f32, tag="den", name=f"dn_{h}_{qh}_{qs}")
                        nc.vector.tensor_copy(
                            out=den[:, :QW2], in_=av[DH : DH + 1, :QW2]
                        )
                        rf = rsm.tile([1, 512], f32, tag="rf", name=f"rf_{h}_{qh}_{qs}")
                        nc.vector.reciprocal_approx_fast(
                            out=rf[:, :QW2], in_=den[:, :QW2]
                        )
                        rb = rbp.tile([64, 512], f32)
                        # replicate across partitions on the (idle) GpSimd
                        # engine: SBUF APs can't stride-0 the partition dim,
                        # so a DMA can't do this and the PE K=1 matmul
                        # broadcast cost ~430ns PE + ~620ns DVE per chain
                        nc.gpsimd.partition_broadcast(rb[:, :QW2], rf[0:1, :QW2])
                        pending_div.append((h, qh, qs, rb, av))

            def finish_div(h, qh, qs, rb, av):
                # a 1-element DVE touch observes the broadcast-DMA completion
                # so the multiply itself needs only the PE wait
                nc.vector.tensor_copy(out=touch[0:1, 5:6], in_=rb[0:1, 0:1])
                col = qs * QW2
                if h % 2 == 0:
                    dst = y_pair[(h // 2, qh)][0:64, col : col + QW2]
                else:
                    dst = yTodd[(h, qh)][:, col : col + QW2]
                nc.vector.tensor_mul(
                    out=dst,
                    in0=av[0:DH, :QW2],
                    in1=rb[:, :QW2],
                )
                if h % 2 == 1:
                    # move the odd head's slice into the pair tile's high half
                    nc.sync.dma_start(
                        y_pair[(h // 2, qh)][64:128, col : col + QW2],
                        yTodd[(h, qh)][:, col : col + QW2],
                    )

            # ---- out-projection chunk emitter (partial, this core's heads) ----
            # PSUM comes from the scores pool (same tag = same slots); early
            # chunks are interleaved into the qh=1 attention phases.
            def emit_outproj(mq, dma_eng=None, act_copy=False):
                opt = ps_sc.tile([128, HW2], f32, tag="spt", name=f"op_{mq}")
                mqh, mcol = (mq * 128) // HW2, (mq * 128) % HW2
                for pp in range(2):
                    nc.tensor.matmul(
                        opt[:, :512],
                        y_pair[(pp, mqh)][:, mcol : mcol + 128],
                        wo2_sb[:, pp, :],
                        start=(pp == 0),
                        stop=(pp == 1),
                    )
                ot = outp.tile([128, 512], bf16)
                # tail chunks copy on the (then-idle) ACT engine so the DVE
                # doesn't pace the PSUM slot recycling; in-loop chunks use
                # DVE (ACT is the bottleneck mid-attention)
                if act_copy:
                    nc.scalar.copy(out=ot[:], in_=opt[:, :512])
                else:
                    nc.vector.tensor_copy(out=ot[:], in_=opt[:, :512])
                (dma_eng or nc.sync).dma_start(out[ts(mq, 128), :], ot[:])

            # Two scores PSUM tiles per step (one per head of the pair, 2
            # banks each) from a bufs=2 pool: head-A's scores of step s+1
            # only wait on exp-A(s) — exp and scores ping-pong with full
            # ACT overlap instead of serializing on one tile.
            for s in range(NSTEP):
                p, qh, g = decode(s)
                # all qh=0 divisions land by step 35 (2 phases + finisher
                # drain), so the first 8 out-proj chunks interleave into the
                # qh=1 phases, one per 3 steps, at the step TOP: the scores
                # pool slot it takes then has a full step of slack before
                # the next scores allocation needs it
                if s == 16:  # PE observes wo2's DMA well before out-proj
                    nc.tensor.ldweights(wo2_sb[:, 0, 0:128])
                if s >= 38 and (s - 38) % 3 == 0 and next_mq[0] < 8:
                    emit_outproj(next_mq[0])
                    next_mq[0] += 1
                if s < 8:  # drip-feed remaining projection work
                    for _ in range(2):
                        if v_backlog:
                            v_backlog.pop(0)()
                        if proj_backlog:
                            proj_backlog.pop(0)()
                # The pair's scores MMs are INTERLEAVED (A,B,A,B): heads
                # A/B occupy disjoint PE row-halves (tile_position auto
                # (0,0)/(64,0)), so each MM's weight load hoists over the
                # other head's in-flight MM instead of serializing
                et = expp.tile([128, T], f16)
                if p == 0:
                    knt, koff = (g * 128) // QW, (g * 128) % QW
                spts = [
                    ps_sc.tile([128, HW2], f32, tag="spt", name=f"spt_{s}_{_h}")
                    for _h in range(2)
                ]
                for qs in range(NQS):
                    for h2 in range(2):
                        hb = h2 * 64
                        if p == 0:
                            lhsT = kTn[knt][hb : hb + 64, koff : koff + 128]
                            rhs = qT0h[qh][
                                hb : hb + 64, qs * QW2 : (qs + 1) * QW2
                            ]
                        else:
                            lhsT = kT1[hb : hb + 64, ts(g, 128)]
                            rhs = qT1[
                                hb : hb + 64,
                                qh * HW2 + qs * QW2 : qh * HW2 + (qs + 1) * QW2,
                            ]
                        nc.tensor.matmul(
                            spts[h2][:, qs * QW2 : (qs + 1) * QW2],
                            lhsT,
                            rhs,
                            start=True,
                            stop=True,
                        )
                for h2 in range(2):
                    nc.scalar.activation(
                        out=et[:, h2 * HW2 : (h2 + 1) * HW2],
                        in_=spts[h2][:],
                        func=mybir.ActivationFunctionType.Exp,
                        bias=exp_bias[:],
                        scale=1.0 / 8.0,
                    )
                expT[s] = et
                # Phase-start steps (no AVs yet, catch-up delayed) leave PE
                # micro-idle gaps that trip the HAM clock-gate back to K=4/8
                # (measured 10-75us throttle latches starting exactly at the
                # phase-0/1 boundary). A dummy ldweights that WAITS on this
                # step's exp executes mid-gap and keeps the activity monitor
                # fed. ~107ns each, only on the 4 sparse steps per phase.
                if s >= 16 and s % KT < 4:
                    nc.tensor.ldweights(et[:, 0:128])
                    nc.tensor.ldweights(et[:, HW2 : HW2 + 128])
                # AV matmuls AFTER the step's scores: the exp->next-scores
                # PSUM ping-pong then overlaps the AV block instead of
                # serializing behind it (scores(s+1) needs exp(s) retired;
                # placing AV between them absorbs the exp latency)
                for aq in av_sched.pop(s, []):
                    av_step(aq)
                # division finishers at the END of the step: their broadcast
                # matmul waits on a DVE reciprocal, and at the head of the
                # step it would stall the PE stream ahead of independent
                # scores/AV work (measured 2.6us ACT gaps per phase boundary)
                for _ in range(2):
                    if pending_div:
                        finish_div(*pending_div.pop(0))
                if s == 8:
                    # anything not drip-fed (shouldn't happen at T=2048)
                    while v_backlog:
                        v_backlog.pop(0)()
                    while proj_backlog:
                        proj_backlog.pop(0)()
            for s in sorted(av_sched):
                for _ in range(2):
                    if pending_div:
                        finish_div(*pending_div.pop(0))
                for aq in av_sched[s]:
                    av_step(aq)
            av_sched.clear()

            # ---- tail: remaining divisions + second-half out-projection ----
            # finishers FIRST: a chunk emitted before a finisher it reads
            # would cycle through the in-order DVE stream (deadlock)
            while pending_div:
                finish_div(*pending_div.pop(0))
            for mq in range(next_mq[0], T // 128):
                emit_outproj(
                    mq,
                    dma_eng=nc.scalar if mq % 2 else nc.sync,
                    act_copy=(mq % 2 == 1),
                )

    nc.finalize()
    return nc


def make_in_maps(x, W_attn, b_attn, W_out):
    """Shard full inputs across 8 cores: core c = (batch c//2, head-half c%2).

    Everything is pre-swizzled into partition-major layouts so device DMA
    descriptors are 2-4KB contiguous per partition:
      xT  [p, chunk, ko, t]  (channel = ko*128 + p, token = chunk*512 + t)
      w*  [p, ko, m]         (input channel = ko*128 + p)
      b*  [p, o]             (channel = o*128 + p)
      wo  [p, pr, n]         (y channel = pr*128 + p)
    """
    bf = ml_dtypes.bfloat16
    in_maps = []
    for c in range(N_CORES):
        b, hh = divmod(c, 2)
        sl = slice(hh * HPC * DH, (hh + 1) * HPC * DH)  # channel slice (256)

        def wsw(w):  # [512, 256] -> [128, 4, 256]
            return np.ascontiguousarray(
                w.reshape(4, 128, HPC * DH).transpose(1, 0, 2)
            ).astype(bf)

        xt = x[b].T  # [C, T]
        in_maps.append(
            {
                "xT": np.ascontiguousarray(
                    xt.reshape(4, 128, 4, 512).transpose(1, 2, 0, 3)
                ).astype(bf),
                "wq": wsw(W_attn[:, 0 * C :][:, sl]),
                "wk": wsw(W_attn[:, 1 * C :][:, sl]),
                "wv": wsw(W_attn[:, 2 * C :][:, sl]),
                "bq": np.ascontiguousarray(
                    b_attn[0 * C :][sl].reshape(2, 128).T, dtype=np.float32
                ),
                "bk": np.ascontiguousarray(
                    b_attn[1 * C :][sl].reshape(2, 128).T, dtype=np.float32
                ),
                "bv": np.ascontiguousarray(b_attn[2 * C :][sl], dtype=np.float32),
                "wo": np.ascontiguousarray(
                    W_out[sl, :].reshape(2, 128, C).transpose(1, 0, 2)
                ).astype(bf),
            }
        )
    return in_maps


def kernel(x, W_attn, b_attn, W_out, b_out, _trace=False):
    from concourse.bass_utils import run_bass_kernel_spmd

    x = np.asarray(x, dtype=np.float32)
    W_attn = np.asarray(W_attn, dtype=np.float32)
    b_attn = np.asarray(b_attn, dtype=np.float32)
    W_out = np.asarray(W_out, dtype=np.float32)
    b_out = np.asarray(b_out, dtype=np.float32)

    key = T_FULL
    if key not in _prog_cache:
        _prog_cache[key] = build_nc(T_FULL)
    nc = _prog_cache[key]

    in_maps = make_in_maps(x, W_attn, b_attn, W_out)
    res = run_bass_kernel_spmd(nc, in_maps, list(range(N_CORES)), trace=_trace)

    out = np.empty((B, T_FULL, C), dtype=np.float32)
    for b in range(B):
        out[b] = (
            res.results[2 * b]["out"].astype(np.float32)
            + res.results[2 * b + 1]["out"].astype(np.float32)
            + b_out
        )
    if _trace:
        kernel.last_exec_time_ns = res.exec_time_ns
        kernel.last_results = res
    return out

